# revision 1
# baseline (speedup 1.0000x reference)
"""Two-layer GAT (PyG semantics) on 8 Trainium2 NeuronCores.

Strategy (graph/data parallel by destination node, per the sharding hint):
  * Host: add self loops; assign nodes to 8 cores (pass 1, balancing edge
    counts), then pack each core's nodes into 49 blocks of 128 "slots"
    (pass 2) so each block's incoming edges fit TA tiles whose src lives on
    cores 0..3 ("half A" of the gathered node table) and TB tiles from
    cores 4..7 ("half B").  The A/B split exists because the bulk-gather
    instruction (dma_gather) takes int16 row indices, so one gather can only
    address 32768 rows; the table is split at row 25088.
  * Device phase A: hcat1 = xT.T @ [W1 | W1@Asrc1 | W1@Adst1 | 0pad] (per-core
    node shard, 320 f32 per row = the 256-byte-multiple row stride dma_gather
    needs), AllGather -> full [50176, 320] node table on every core
    (cols 0:256 = h, 256:264 = alpha_src, 264:272 = alpha_dst).
  * Device phase B (layer-1 edges, per block): dma_gather of hcat1[src] rows
    (one gather per half), dma_gather of the dst alpha terms from the core's
    OWN shard (local indices), p = exp(leaky_relu(s+d)), build a one-hot
    selection matrix B[e, dst_local] on the DVE, scale the gathered rows by p
    in place, and accumulate  out[dst] = sum_e p_e * h[src_e]  plus the
    softmax denominator (an appended column of p) with PE matmuls
    B.T @ [p*h | p] into PSUM.  Softmax normalization = one divide by the
    accumulated denominator at the end (mathematically identical to the
    reference's max-subtracted softmax; logits are O(1) so exp cannot
    overflow).  Dummy padding edges point at a reserved node row whose
    alpha_src is -1e9, making their p exactly 0.  Then ELU and a PE
    transpose build h2T for the next layer.
  * Phase C/D: same again for layer 2 (40 features, 1 head) -> per-core out.
  * Host: concatenate core outputs, inverse-permute, add b2.
"""

import os

import numpy as np

# ---------------- geometry (hardcoded for nn_GAT_51694226374713) ------------
N_NODES = 50000
N_EDGES = 800000
N_CORES = 8
NB = 49                    # dst blocks per core
PB = 128                   # dst nodes (slots) per block
SLOTS = NB * PB            # 6272 node slots per core
V = N_CORES * SLOTS        # 50176 rows in the gathered node tables
TA = int(os.environ.get("GAT_TA", "10"))   # edge tiles from table half A
TB = int(os.environ.get("GAT_TB", "10"))   # edge tiles from table half B
TT = TA + TB
F1 = 256                   # input features
H1, C1 = 8, 32             # layer-1 heads x channels
T1W = 320                  # hcat1 row width (f32): h | s | d | pad, 1280 B
NCLS = 40
T2W = 64                   # hcat2 row width: h2(40) | s(1) | d(1) | pad, 256 B
SPLIT = (N_CORES // 2) * SLOTS   # table half boundary (row 25088)
DUMMY_ROW = SLOTS - 1      # local row 6271 on every core; s == -1e9 there
NEG_SLOPE = 0.2
NEG_BIG = -1.0e9

_CACHE: dict = {}


def _set_geometry(n_nodes, n_edges, n_cores, nb, ta, tb):
    """Override problem geometry (used only by small-scale sim tests)."""
    global N_NODES, N_EDGES, N_CORES, NB, SLOTS, V, TA, TB, TT, SPLIT, DUMMY_ROW
    N_NODES, N_EDGES, N_CORES, NB, TA, TB = n_nodes, n_edges, n_cores, nb, ta, tb
    TT = TA + TB
    SLOTS = NB * PB
    V = N_CORES * SLOTS
    SPLIT = (N_CORES // 2) * SLOTS
    DUMMY_ROW = SLOTS - 1
    _CACHE.clear()


# ============================ host preprocessing ============================

def _greedy_pack(items, weights_list, caps_list, slot_caps):
    """Place items (ordered) into bins; weights_list/caps_list are parallel
    lists of per-item weight arrays and per-bin capacity arrays.  Returns
    (bin_of_item, slot_of_item).  Greedy: emptiest bin (by total weight)
    first, skipping bins where any cap or the slot cap would overflow."""
    import heapq

    n_bins = len(slot_caps)
    used = [np.zeros(n_bins, dtype=np.int64) for _ in weights_list]
    slots_used = np.zeros(n_bins, dtype=np.int64)
    total = np.zeros(n_bins, dtype=np.int64)
    bin_of = {}
    slot_of = {}
    heap = [(0, b) for b in range(n_bins)]
    heapq.heapify(heap)
    for it in items:
        ws = [w[it] for w in weights_list]
        stash = []
        while True:
            if not heap:
                raise RuntimeError("packing failed; increase GAT_TA/GAT_TB")
            t, b = heapq.heappop(heap)
            if t != total[b]:
                continue  # stale
            if slots_used[b] >= slot_caps[b]:
                continue  # permanently full
            if any(
                used[k][b] + ws[k] > caps_list[k][b] for k in range(len(ws))
            ):
                stash.append((t, b))
                continue
            bin_of[it] = b
            slot_of[it] = slots_used[b]
            slots_used[b] += 1
            for k in range(len(ws)):
                used[k][b] += ws[k]
            total[b] += sum(ws)
            heapq.heappush(heap, (int(total[b]), b))
            break
        for item in stash:
            heapq.heappush(heap, item)
    return bin_of, slot_of


def _wrap_idx(lin):
    """Linear index array [n] -> dma_gather layout [128, n//16] int16."""
    n = lin.size
    assert n % 16 == 0
    w = lin.reshape(n // 16, 16).T.astype(np.int16)  # [16, n/16]
    return np.ascontiguousarray(np.tile(w, (8, 1)))  # [128, n/16]


def _pack_graph(src, dst):
    """Assign nodes to (core, block, slot); route edges.

    Returns perm_row [N], and per-core index arrays for the device:
      idxA  [NC, NB, 128, TA*8] i16 -- src rows in [0, SPLIT), half-A edges
      idxB  [NC, NB, 128, TB*8] i16 -- src rows - SPLIT, half-B edges
      idxD  [NC, NB, 128, TT*8] i16 -- dst local rows in [0, SLOTS)
      dstloc [NC, NB, 128, TT] f32 -- dst slot within block (0..127)
    """
    deg = np.bincount(dst, minlength=N_NODES)

    # ---- pass 1: nodes -> cores, balancing total in-edges ----
    order = np.argsort(-deg, kind="stable")
    core_slot_caps = np.full(N_CORES, SLOTS - 1, dtype=np.int64)  # reserve dummy
    core_of, _ = _greedy_pack(
        order,
        [deg],
        [np.full(N_CORES, 1 << 60, dtype=np.int64)],
        core_slot_caps,
    )
    node_core = np.empty(N_NODES, dtype=np.int64)
    for nd, c in core_of.items():
        node_core[nd] = c

    # src half of each edge is now fixed: A = cores [0, NC/2)
    half_b_src = node_core[src] >= (N_CORES // 2)
    degA = np.bincount(dst[~half_b_src], minlength=N_NODES)
    degB = np.bincount(dst[half_b_src], minlength=N_NODES)

    # ---- pass 2: per core, nodes -> blocks with per-half edge caps ----
    node_bin = np.empty(N_NODES, dtype=np.int64)
    node_slot = np.empty(N_NODES, dtype=np.int64)
    for c in range(N_CORES):
        nodes_c = np.where(node_core == c)[0]
        ordc = nodes_c[np.argsort(-(deg[nodes_c]), kind="stable")]
        slot_caps = np.full(NB, PB, dtype=np.int64)
        slot_caps[NB - 1] = PB - 1  # dummy slot
        bin_of, slot_of = _greedy_pack(
            ordc,
            [degA, degB],
            [
                np.full(NB, TA * PB, dtype=np.int64),
                np.full(NB, TB * PB, dtype=np.int64),
            ],
            slot_caps,
        )
        for nd in ordc:
            node_bin[nd] = c * NB + bin_of[nd]
            node_slot[nd] = slot_of[nd]

    core_of_bin = np.arange(N_CORES * NB) // NB
    block_of_bin = np.arange(N_CORES * NB) % NB
    perm_row = (
        core_of_bin[node_bin] * SLOTS + block_of_bin[node_bin] * PB + node_slot
    ).astype(np.int64)

    # ---- edge routing: per (bin, half), sorted by src row ----
    n_bins = N_CORES * NB
    ebin = node_bin[dst]
    src_row_e = perm_row[src]
    dst_row_e = perm_row[dst]
    # order: (bin, half, src_row)
    keyhalf = half_b_src.astype(np.int64)
    sort_idx = np.lexsort((src_row_e, keyhalf, ebin))
    ebin_s = ebin[sort_idx]
    half_s = keyhalf[sort_idx]
    src_s = src_row_e[sort_idx]
    dst_s = dst_row_e[sort_idx]

    capA, capB = TA * PB, TB * PB
    DUMMY_A = DUMMY_ROW                      # global row, in half A
    DUMMY_B = SPLIT + DUMMY_ROW              # core NC/2's dummy row

    # positions within (bin, half) groups
    grp = ebin_s * 2 + half_s
    counts = np.bincount(grp, minlength=n_bins * 2)
    cA = counts[0::2]
    cB = counts[1::2]
    assert cA.max() <= capA and cB.max() <= capB, (cA.max(), cB.max())
    starts = np.zeros(n_bins * 2 + 1, dtype=np.int64)
    np.cumsum(counts, out=starts[1:])
    pos = np.arange(ebin_s.size) - starts[grp]

    # j position within the block's TT*PB edge list
    j = np.where(half_s == 0, pos, capA + pos)

    srcA = np.full((n_bins, capA), DUMMY_A, dtype=np.int64)
    srcB = np.full((n_bins, capB), DUMMY_B - SPLIT, dtype=np.int64)
    dstl = np.full((n_bins, TT * PB), DUMMY_ROW, dtype=np.int64)
    dslot = np.zeros((n_bins, TT * PB), dtype=np.int64)

    mA = half_s == 0
    srcA[ebin_s[mA], pos[mA]] = src_s[mA]
    srcB[ebin_s[~mA], pos[~mA]] = src_s[~mA] - SPLIT
    dstl[ebin_s, j] = dst_s % SLOTS
    dslot[ebin_s, j] = dst_s % PB

    idxA = np.stack(
        [_wrap_idx(srcA[b]) for b in range(n_bins)]
    ).reshape(N_CORES, NB, 128, capA // 16)
    idxB = np.stack(
        [_wrap_idx(srcB[b]) for b in range(n_bins)]
    ).reshape(N_CORES, NB, 128, capB // 16)
    idxD = np.stack(
        [_wrap_idx(dstl[b]) for b in range(n_bins)]
    ).reshape(N_CORES, NB, 128, (TT * PB) // 16)
    # dstloc in (p, t) layout: j = t*128 + p
    dstloc = np.ascontiguousarray(
        dslot.reshape(N_CORES, NB, TT, PB).transpose(0, 1, 3, 2)
    ).astype(np.float32)
    return perm_row, idxA, idxB, idxD, dstloc


def _expand_heads(a):
    """[H, C] attention vector -> block-diagonal [H*C, H] matrix."""
    h, c = a.shape
    m = np.zeros((h * c, h), dtype=np.float32)
    for i in range(h):
        m[i * c:(i + 1) * c, i] = a[i]
    return m


# ============================ device program ================================

def _build_program():
    import concourse.bacc as bacc
    import concourse.bass as bass
    import concourse.mybir as mybir
    import concourse.tile as tile

    f32 = mybir.dt.float32
    i16 = mybir.dt.int16
    Alu = mybir.AluOpType
    Act = mybir.ActivationFunctionType

    nc = bacc.Bacc(
        "TRN2", target_bir_lowering=False, debug=False, num_devices=N_CORES
    )

    # ---- kernel I/O ----
    xT = nc.dram_tensor("xT", [F1, SLOTS], f32, kind="ExternalInput")
    w1cat = nc.dram_tensor("w1cat", [F1, T1W], f32, kind="ExternalInput")
    w2cat = nc.dram_tensor("w2cat", [F1, T2W], f32, kind="ExternalInput")
    iota_in = nc.dram_tensor("iota_row", [PB, PB], f32, kind="ExternalInput")
    ident_in = nc.dram_tensor("ident", [PB, PB], f32, kind="ExternalInput")
    idxA_in = nc.dram_tensor(
        "idxA", [NB, PB, TA * PB // 16], i16, kind="ExternalInput"
    )
    idxB_in = nc.dram_tensor(
        "idxB", [NB, PB, TB * PB // 16], i16, kind="ExternalInput"
    )
    idxD_in = nc.dram_tensor(
        "idxD", [NB, PB, TT * PB // 16], i16, kind="ExternalInput"
    )
    dstloc_in = nc.dram_tensor("dstloc", [NB, PB, TT], f32, kind="ExternalInput")
    out_dev = nc.dram_tensor("out_dev", [SLOTS, NCLS], f32, kind="ExternalOutput")

    debug_taps = bool(int(os.environ.get("GAT_DEBUG", "0")))
    stop = int(os.environ.get("GAT_STOP", "0"))  # 0 = full program
    dbg = {}
    if debug_taps:
        for nm, shp in [
            ("hcat1own", [PB, T1W]),
            ("hcat1all", [PB, T1W]),
            ("G", [PB, TT * T1W]),
            ("Dg", [PB, TT * T2W]),
            ("p", [PB, TT * H1]),
            ("po", [PB, F1 + H1]),
            ("h2", [PB, F1]),
        ]:
            dbg[nm] = nc.dram_tensor(f"dbg_{nm}", shp, f32, kind="ExternalOutput")

    # ---- internal DRAM ----
    aspace = "Shared" if N_CORES > 4 else "Local"
    if os.environ.get("GAT_AG_LOCAL") == "1":
        aspace = "Local"
    hcat1_own = nc.dram_tensor("hcat1_own", [SLOTS, T1W], f32, kind="Internal")
    hcat1_all = nc.dram_tensor(
        "hcat1_all", [V, T1W], f32, kind="Internal", addr_space=aspace
    )
    hcat2_own = nc.dram_tensor("hcat2_own", [SLOTS, T2W], f32, kind="Internal")
    hcat2_all = nc.dram_tensor(
        "hcat2_all", [V, T2W], f32, kind="Internal", addr_space=aspace
    )

    groups = [list(range(N_CORES))]
    NH = SPLIT  # rows per table half


    with tile.TileContext(nc) as tc:
        with (
            tc.tile_pool(name="persist", bufs=1) as pp,
            tc.tile_pool(name="sb", bufs=2) as sb,
            tc.tile_pool(name="psum", bufs=2, space="PSUM") as pmm,
        ):
            # ---------------- persistent tiles ----------------
            iota_sb = pp.tile([PB, PB], f32, tag="iota")
            nc.sync.dma_start(out=iota_sb[:], in_=iota_in[:, :])
            ident_sb = pp.tile([PB, PB], f32, tag="ident")
            nc.sync.dma_start(out=ident_sb[:], in_=ident_in[:, :])
            negbig_sb = pp.tile([1, H1], f32, tag="negbig")
            nc.gpsimd.memset(negbig_sb[:], NEG_BIG)

            w1_sb = [
                pp.tile([PB, T1W], f32, tag=f"w1_{k}", name=f"w1_sb{k}")
                for k in range(2)
            ]
            for k in range(2):
                nc.sync.dma_start(out=w1_sb[k][:], in_=w1cat[k * PB:(k + 1) * PB, :])
            w2_sb = [
                pp.tile([PB, T2W], f32, tag=f"w2_{k}", name=f"w2_sb{k}")
                for k in range(2)
            ]
            for k in range(2):
                nc.sync.dma_start(out=w2_sb[k][:], in_=w2cat[k * PB:(k + 1) * PB, :])

            # xT and h2T share the two big slots (xT dead before h2T born)
            xT_sb = [
                pp.tile([PB, SLOTS], f32, tag=f"big{k}", name=f"xT_sb{k}")
                for k in range(2)
            ]
            for k in range(2):
                nc.sync.dma_start(out=xT_sb[k][:], in_=xT[k * PB:(k + 1) * PB, :])

            # ---------------- phase A: hcat1 = x @ W1cat ----------------
            with nc.named_scope("phaseA"):
                for nb in range(NB):
                    ps = pmm.tile([PB, T1W], f32, tag="mm")
                    for k in range(2):
                        nc.tensor.matmul(
                            out=ps[:],
                            lhsT=xT_sb[k][:][:, nb * PB:(nb + 1) * PB],
                            rhs=w1_sb[k][:],
                            start=(k == 0),
                            stop=(k == 1),
                        )
                    hc = sb.tile([PB, T1W], f32, tag="hc1")
                    nc.scalar.copy(out=hc[:], in_=ps[:])
                    nc.sync.dma_start(
                        out=hcat1_own[nb * PB:(nb + 1) * PB, :], in_=hc[:]
                    )
                # dummy row: s = -1e9 so dummy edges get p = exp(-inf) = 0
                nc.sync.dma_start(
                    out=hcat1_own[DUMMY_ROW:DUMMY_ROW + 1, F1:F1 + H1],
                    in_=negbig_sb[:1, :],
                )

            with nc.named_scope("allgather1"):
                nc.gpsimd.collective_compute(
                    "AllGather",
                    mybir.AluOpType.bypass,
                    replica_groups=groups,
                    ins=[hcat1_own[:, :].opt()],
                    outs=[hcat1_all[:, :].opt()],
                )

            if debug_taps:
                t1 = sb.tile([PB, T1W], f32, tag="dbg1")
                nc.sync.dma_start(out=t1[:], in_=hcat1_own[0:PB, :])
                nc.sync.dma_start(out=dbg["hcat1own"][:, :], in_=t1[:])
                t2 = sb.tile([PB, T1W], f32, tag="dbg2")
                nc.sync.dma_start(out=t2[:], in_=hcat1_all[SLOTS:SLOTS + PB, :])
                nc.sync.dma_start(out=dbg["hcat1all"][:, :], in_=t2[:])

            # ---------------- phase B: layer-1 edges ----------------
            h2T_sb = [
                pp.tile([PB, SLOTS], f32, tag=f"big{k}", name=f"h2T_sb{k}")
                for k in range(2)
            ]
            with nc.named_scope("edges1"):
                for b in range(NB if stop != 1 else 0):
                    iA = sb.tile([PB, TA * PB // 16], i16, tag="iA")
                    nc.sync.dma_start(out=iA[:], in_=idxA_in[b, :, :])
                    iB = sb.tile([PB, TB * PB // 16], i16, tag="iB")
                    nc.sync.dma_start(out=iB[:], in_=idxB_in[b, :, :])
                    iD = sb.tile([PB, TT * PB // 16], i16, tag="iD")
                    nc.sync.dma_start(out=iD[:], in_=idxD_in[b, :, :])
                    dloc = sb.tile([PB, TT], f32, tag="dloc")
                    nc.sync.dma_start(out=dloc[:], in_=dstloc_in[b, :, :])

                    # gather hcat1[src]: half A -> chunks [0, TA), B -> rest
                    G = sb.tile([PB, TT * T1W], f32, tag="G")
                    G3 = G[:].rearrange("p (t f) -> p t f", t=TT)
                    nc.gpsimd.dma_gather(
                        out_ap=G3[:, 0:TA, :],
                        in_ap=hcat1_all[0:NH, :],
                        idxs_ap=iA[:],
                        num_idxs=TA * PB,
                        num_idxs_reg=TA * PB,
                        elem_size=T1W,
                        single_packet=False,
                    )
                    nc.gpsimd.dma_gather(
                        out_ap=G3[:, TA:TT, :],
                        in_ap=hcat1_all[NH:V, :],
                        idxs_ap=iB[:],
                        num_idxs=TB * PB,
                        num_idxs_reg=TB * PB,
                        elem_size=T1W,
                        single_packet=False,
                    )
                    # gather [s|d|pad] (cols 256:320) of hcat1_own[dst_local]
                    Dg = sb.tile([PB, TT * T2W], f32, tag="Dg")
                    Dg3 = Dg[:].rearrange("p (t f) -> p t f", t=TT)
                    nc.gpsimd.dma_gather(
                        out_ap=Dg3,
                        in_ap=hcat1_own[:, F1:F1 + T2W],
                        idxs_ap=iD[:],
                        num_idxs=TT * PB,
                        num_idxs_reg=TT * PB,
                        elem_size=T2W,
                        elem_step=T1W,
                        single_packet=False,
                    )

                    if stop == 2:
                        if debug_taps and b == 0:
                            nc.sync.dma_start(out=dbg["G"][:, :], in_=G[:])
                            nc.sync.dma_start(out=dbg["Dg"][:, :], in_=Dg[:])
                        continue
                    # logits -> p = exp(leaky_relu(s_src + d_dst))
                    lg = sb.tile([PB, TT * H1], f32, tag="lg")
                    lg3 = lg[:].rearrange("p (t h) -> p t h", t=TT)
                    nc.vector.tensor_tensor(
                        out=lg3,
                        in0=G3[:, :, F1:F1 + H1],
                        in1=Dg3[:, :, H1:2 * H1],
                        op=Alu.add,
                    )
                    lg2 = sb.tile([PB, TT * H1], f32, tag="lg2")
                    nc.vector.tensor_scalar_mul(
                        out=lg2[:], in0=lg[:], scalar1=NEG_SLOPE
                    )
                    nc.vector.tensor_tensor(
                        out=lg[:], in0=lg[:], in1=lg2[:], op=Alu.max
                    )
                    p = sb.tile([PB, TT * H1], f32, tag="p")
                    nc.scalar.activation(out=p[:], in_=lg[:], func=Act.Exp)
                    p3 = p[:].rearrange("p (t h) -> p t h", t=TT)

                    # selection matrix B[e, (t, d)] = (dstloc[e,t] == d)
                    Bm = sb.tile([PB, TT * PB], f32, tag="Bm")
                    Bm3 = Bm[:].rearrange("p (t d) -> p t d", t=TT)
                    nc.vector.tensor_tensor(
                        out=Bm3,
                        in0=dloc[:][:, :, None].broadcast_to([PB, TT, PB]),
                        in1=iota_sb[:][:, None, :].broadcast_to([PB, TT, PB]),
                        op=Alu.is_equal,
                    )

                    # in-place: G[:, :, 0:256] *= p ; G[:, :, 256:264] = p
                    out4 = G3[:, :, 0:F1].rearrange("p t (h c) -> p t h c", h=H1)
                    nc.vector.tensor_tensor(
                        out=out4,
                        in0=out4,
                        in1=p3[:, :, :, None].broadcast_to([PB, TT, H1, C1]),
                        op=Alu.mult,
                    )
                    nc.vector.tensor_copy(out=G3[:, :, F1:F1 + H1], in_=p3)

                    # accumulate over edge tiles:  out1[d] = B.T @ [p*h | p]
                    po = pmm.tile([PB, F1 + H1], f32, tag="mm")
                    for t in range(TT):
                        nc.tensor.matmul(
                            out=po[:],
                            lhsT=Bm[:][:, t * PB:(t + 1) * PB],
                            rhs=G[:][:, t * T1W:t * T1W + F1 + H1],
                            start=(t == 0),
                            stop=(t == TT - 1),
                        )

                    if debug_taps and b == 0:
                        nc.sync.dma_start(out=dbg["G"][:, :], in_=G[:])
                        nc.sync.dma_start(out=dbg["Dg"][:, :], in_=Dg[:])
                        nc.sync.dma_start(out=dbg["p"][:, :], in_=p[:])
                        pot = sb.tile([PB, F1 + H1], f32, tag="dbgpo")
                        nc.vector.tensor_copy(out=pot[:], in_=po[:])
                        nc.sync.dma_start(out=dbg["po"][:, :], in_=pot[:])

                    if stop == 3:
                        continue
                    # normalize, ELU
                    den = sb.tile([PB, H1], f32, tag="den")
                    nc.vector.tensor_copy(out=den[:], in_=po[:][:, F1:F1 + H1])
                    dfx = sb.tile([PB, H1], f32, tag="dfx")
                    nc.vector.tensor_scalar(
                        out=dfx[:], in0=den[:], scalar1=0.0, scalar2=None,
                        op0=Alu.is_equal,
                    )
                    nc.vector.tensor_tensor(
                        out=dfx[:], in0=den[:], in1=dfx[:], op=Alu.add
                    )
                    rden = sb.tile([PB, H1], f32, tag="rden")
                    nc.vector.reciprocal(out=rden[:], in_=dfx[:])

                    o1 = sb.tile([PB, F1], f32, tag="o1")
                    o13 = o1[:].rearrange("p (h c) -> p h c", h=H1)
                    nc.vector.tensor_tensor(
                        out=o13,
                        in0=po[:][:, 0:F1].rearrange("p (h c) -> p h c", h=H1),
                        in1=rden[:][:, :, None].broadcast_to([PB, H1, C1]),
                        op=Alu.mult,
                    )
                    # elu(x) = max(x,0) - 1 + exp(min(x,0))
                    mneg = sb.tile([PB, F1], f32, tag="mneg")
                    nc.vector.tensor_scalar_min(out=mneg[:], in0=o1[:], scalar1=0.0)
                    eneg = sb.tile([PB, F1], f32, tag="eneg")
                    nc.scalar.activation(out=eneg[:], in_=mneg[:], func=Act.Exp)
                    h2a = sb.tile([PB, F1], f32, tag="h2a")
                    nc.vector.tensor_scalar(
                        out=h2a[:], in0=o1[:], scalar1=0.0, scalar2=-1.0,
                        op0=Alu.max, op1=Alu.add,
                    )
                    h2 = sb.tile([PB, F1], f32, tag="h2")
                    nc.vector.tensor_tensor(
                        out=h2[:], in0=h2a[:], in1=eneg[:], op=Alu.add
                    )
                    if debug_taps and b == 0:
                        nc.sync.dma_start(out=dbg["h2"][:, :], in_=h2[:])

                    # transpose h2 into h2T for the layer-2 matmul
                    for k in range(2):
                        pt = pmm.tile([PB, PB], f32, tag="tr")
                        nc.tensor.transpose(
                            out=pt[:],
                            in_=h2[:][:, k * PB:(k + 1) * PB],
                            identity=ident_sb[:],
                        )
                        nc.scalar.copy(
                            out=h2T_sb[k][:][:, b * PB:(b + 1) * PB], in_=pt[:]
                        )

            # ---------------- phase C: hcat2 = h2 @ W2cat ----------------
            with nc.named_scope("phaseC"):
                for nb in range(NB if stop in (0, 5) else 0):
                    ps = pmm.tile([PB, T2W], f32, tag="mm")
                    for k in range(2):
                        nc.tensor.matmul(
                            out=ps[:],
                            lhsT=h2T_sb[k][:][:, nb * PB:(nb + 1) * PB],
                            rhs=w2_sb[k][:],
                            start=(k == 0),
                            stop=(k == 1),
                        )
                    hc2 = sb.tile([PB, T2W], f32, tag="hc2")
                    nc.scalar.copy(out=hc2[:], in_=ps[:])
                    nc.sync.dma_start(
                        out=hcat2_own[nb * PB:(nb + 1) * PB, :], in_=hc2[:]
                    )
                if stop in (0, 5):
                    nc.sync.dma_start(
                        out=hcat2_own[DUMMY_ROW:DUMMY_ROW + 1, NCLS:NCLS + 1],
                        in_=negbig_sb[:1, :1],
                    )

            with nc.named_scope("allgather2"):
                if stop in (0, 5):
                    nc.gpsimd.collective_compute(
                    "AllGather",
                    mybir.AluOpType.bypass,
                        replica_groups=groups,
                        ins=[hcat2_own[:, :].opt()],
                        outs=[hcat2_all[:, :].opt()],
                    )

            # ---------------- phase D: layer-2 edges ----------------
            with nc.named_scope("edges2"):
                for b in range(NB if stop == 0 else 0):
                    iA = sb.tile([PB, TA * PB // 16], i16, tag="iA")
                    nc.sync.dma_start(out=iA[:], in_=idxA_in[b, :, :])
                    iB = sb.tile([PB, TB * PB // 16], i16, tag="iB")
                    nc.sync.dma_start(out=iB[:], in_=idxB_in[b, :, :])
                    iD = sb.tile([PB, TT * PB // 16], i16, tag="iD")
                    nc.sync.dma_start(out=iD[:], in_=idxD_in[b, :, :])
                    dloc = sb.tile([PB, TT], f32, tag="dloc")
                    nc.sync.dma_start(out=dloc[:], in_=dstloc_in[b, :, :])

                    G2 = sb.tile([PB, TT * T2W], f32, tag="G2")
                    G23 = G2[:].rearrange("p (t f) -> p t f", t=TT)
                    nc.gpsimd.dma_gather(
                        out_ap=G23[:, 0:TA, :],
                        in_ap=hcat2_all[0:NH, :],
                        idxs_ap=iA[:],
                        num_idxs=TA * PB,
                        num_idxs_reg=TA * PB,
                        elem_size=T2W,
                        single_packet=False,
                    )
                    nc.gpsimd.dma_gather(
                        out_ap=G23[:, TA:TT, :],
                        in_ap=hcat2_all[NH:V, :],
                        idxs_ap=iB[:],
                        num_idxs=TB * PB,
                        num_idxs_reg=TB * PB,
                        elem_size=T2W,
                        single_packet=False,
                    )
                    D2 = sb.tile([PB, TT * T2W], f32, tag="D2")
                    D23 = D2[:].rearrange("p (t f) -> p t f", t=TT)
                    nc.gpsimd.dma_gather(
                        out_ap=D23,
                        in_ap=hcat2_own[:, :],
                        idxs_ap=iD[:],
                        num_idxs=TT * PB,
                        num_idxs_reg=TT * PB,
                        elem_size=T2W,
                        single_packet=False,
                    )

                    lg = sb.tile([PB, TT], f32, tag="lgB")
                    lg3 = lg[:].rearrange("p (t h) -> p t h", t=TT)
                    nc.vector.tensor_tensor(
                        out=lg3,
                        in0=G23[:, :, NCLS:NCLS + 1],
                        in1=D23[:, :, NCLS + 1:NCLS + 2],
                        op=Alu.add,
                    )
                    lg2 = sb.tile([PB, TT], f32, tag="lg2B")
                    nc.vector.tensor_scalar_mul(
                        out=lg2[:], in0=lg[:], scalar1=NEG_SLOPE
                    )
                    nc.vector.tensor_tensor(
                        out=lg[:], in0=lg[:], in1=lg2[:], op=Alu.max
                    )
                    p2 = sb.tile([PB, TT], f32, tag="p2")
                    nc.scalar.activation(out=p2[:], in_=lg[:], func=Act.Exp)
                    p23 = p2[:].rearrange("p (t h) -> p t h", t=TT)

                    Bm = sb.tile([PB, TT * PB], f32, tag="Bm")
                    Bm3 = Bm[:].rearrange("p (t d) -> p t d", t=TT)
                    nc.vector.tensor_tensor(
                        out=Bm3,
                        in0=dloc[:][:, :, None].broadcast_to([PB, TT, PB]),
                        in1=iota_sb[:][:, None, :].broadcast_to([PB, TT, PB]),
                        op=Alu.is_equal,
                    )

                    # in-place: G2[:, :, 0:40] *= p2 ; G2[:, :, 40] = p2
                    nc.vector.tensor_tensor(
                        out=G23[:, :, 0:NCLS],
                        in0=G23[:, :, 0:NCLS],
                        in1=p23.broadcast_to([PB, TT, NCLS]),
                        op=Alu.mult,
                    )
                    nc.vector.tensor_copy(out=G23[:, :, NCLS:NCLS + 1], in_=p23)

                    po = pmm.tile([PB, NCLS + 1], f32, tag="mm")
                    for t in range(TT):
                        nc.tensor.matmul(
                            out=po[:],
                            lhsT=Bm[:][:, t * PB:(t + 1) * PB],
                            rhs=G2[:][:, t * T2W:t * T2W + NCLS + 1],
                            start=(t == 0),
                            stop=(t == TT - 1),
                        )

                    den = sb.tile([PB, 1], f32, tag="denB")
                    nc.vector.tensor_copy(out=den[:], in_=po[:][:, NCLS:NCLS + 1])
                    dfx = sb.tile([PB, 1], f32, tag="dfxB")
                    nc.vector.tensor_scalar(
                        out=dfx[:], in0=den[:], scalar1=0.0, scalar2=None,
                        op0=Alu.is_equal,
                    )
                    nc.vector.tensor_tensor(
                        out=dfx[:], in0=den[:], in1=dfx[:], op=Alu.add
                    )
                    rden = sb.tile([PB, 1], f32, tag="rdenB")
                    nc.vector.reciprocal(out=rden[:], in_=dfx[:])

                    ot = sb.tile([PB, NCLS], f32, tag="ot")
                    nc.vector.tensor_tensor(
                        out=ot[:],
                        in0=po[:][:, 0:NCLS],
                        in1=rden[:].broadcast_to([PB, NCLS]),
                        op=Alu.mult,
                    )
                    nc.sync.dma_start(
                        out=out_dev[b * PB:(b + 1) * PB, :], in_=ot[:]
                    )

    nc.compile()
    return nc


# ============================ top-level entry ===============================

def _prepare(inputs):
    x = np.ascontiguousarray(np.asarray(inputs["x"], dtype=np.float32))
    edge_index = np.asarray(inputs["edge_index"], dtype=np.int64)
    w1 = np.asarray(inputs["w1"], dtype=np.float32)
    a_src1 = np.asarray(inputs["a_src1"], dtype=np.float32)
    a_dst1 = np.asarray(inputs["a_dst1"], dtype=np.float32)
    b1 = np.asarray(inputs["b1"], dtype=np.float32)
    w2 = np.asarray(inputs["w2"], dtype=np.float32)
    a_src2 = np.asarray(inputs["a_src2"], dtype=np.float32)
    a_dst2 = np.asarray(inputs["a_dst2"], dtype=np.float32)
    b2 = np.asarray(inputs["b2"], dtype=np.float32)

    assert x.shape == (N_NODES, F1) and edge_index.shape == (2, N_EDGES)
    assert np.all(np.abs(b1) == 0.0), "kernel hardcodes b1 == 0"

    loops = np.arange(N_NODES, dtype=np.int64)
    src = np.concatenate([edge_index[0], loops])
    dst = np.concatenate([edge_index[1], loops])
    perm_row, idxA, idxB, idxD, dstloc = _pack_graph(src, dst)

    w1cat = np.concatenate(
        [
            w1,
            w1 @ _expand_heads(a_src1),
            w1 @ _expand_heads(a_dst1),
            np.zeros((F1, T1W - F1 - 2 * H1), dtype=np.float32),
        ],
        axis=1,
    ).astype(np.float32)
    w2cat = np.concatenate(
        [
            w2,
            w2 @ _expand_heads(a_src2),
            w2 @ _expand_heads(a_dst2),
            np.zeros((F1, T2W - NCLS - 2), dtype=np.float32),
        ],
        axis=1,
    ).astype(np.float32)

    xp = np.zeros((V, F1), dtype=np.float32)
    xp[perm_row] = x
    iota_row = np.broadcast_to(np.arange(PB, dtype=np.float32), (PB, PB)).copy()
    ident = np.eye(PB, dtype=np.float32)

    in_maps = []
    for c in range(N_CORES):
        xT_c = np.ascontiguousarray(xp[c * SLOTS:(c + 1) * SLOTS].T)
        in_maps.append(
            {
                "xT": xT_c,
                "w1cat": w1cat,
                "w2cat": w2cat,
                "iota_row": iota_row,
                "ident": ident,
                "idxA": idxA[c],
                "idxB": idxB[c],
                "idxD": idxD[c],
                "dstloc": dstloc[c],
            }
        )
    return in_maps, perm_row, b2


def _assemble(core_outs, perm_row, b2):
    out_all = np.concatenate(core_outs, axis=0)
    out = out_all[perm_row] + b2[None, :]
    return out.astype(np.float32)


def kernel(**inputs) -> np.ndarray:
    in_maps, perm_row, b2 = _prepare(inputs)

    import concourse.bass_utils as bass_utils

    if "nc" not in _CACHE:
        _CACHE["nc"] = _build_program()
    nc = _CACHE["nc"]

    trace = bool(int(os.environ.get("GAT_TRACE", "0")))
    res = bass_utils.run_bass_kernel_spmd(
        nc,
        in_maps,
        core_ids=list(range(N_CORES)),
        trace=trace,
        trace_cores=list(range(N_CORES)) if trace else None,
        stitch_traces=trace,
    )
    _CACHE["last_results"] = res

    return _assemble([r["out_dev"] for r in res.results], perm_row, b2)



# revision 10
# speedup vs baseline: 2.8942x; 2.8942x over previous
"""Two-layer GAT (PyG semantics) on 8 Trainium2 NeuronCores.

Gather-lean bf16 redesign.  The baseline spent 4.2 of 5.1 ms on GPSIMD Q7
descriptor generation for dma_gather (~8 ns per gathered row, 502k rows
per core).  This version cuts gather indices ~2.5x and moves everything
else off the critical engine:

  * No self-loop edges in the edge list: the self-loop term p_self*h[own]
    is computed per dst block from the locally-stored rows (every node
    slot gets exactly one self loop, including empty slots, which hold
    h=0 and stay harmless).
  * No per-edge dst-alpha gather (the baseline's Dg/D2, 50% of indices).
    Instead alpha_dst is broadcast per block on the PE: a one-partition
    matmul replicates the per-edge dst-slot ids down 128 partitions, DVE
    is_equal against a column-iota builds the transposed one-hot
    BmT[d, e], and BmT.T @ dblk gives alpha_dst per edge slot.
  * bf16 node tables: layer-1 rows [h(256)|s(8)|d(8)] at a 768 B stride
    (vs 1280 B f32), layer-2 rows [h2(40)|s|d] at 256 B.  Gathers move
    real edges only (per-block counts baked at trace time as the max
    across cores; tails are memset + killed by zero one-hot columns).
  * bf16 PE matmuls (1 cycle/row vs 4 for fp32) for edge aggregation
    B.T @ [p*h | p], with f32 PSUM accumulation.

Host: add no self loops; assign nodes to 8 cores balancing in-edges,
pack each core's nodes into 49 blocks of 128 slots; route edges to the
block owning their dst, split by src table half (int16 gather indices
address <=32768 rows; the table splits at row 25088), sorted by src row.
Device: hcat = x @ [W|W@a_src|W@a_dst] per core, AllGather to the full
table, per-block gather + softmax + one-hot-matmul aggregation + ELU,
then the same again for layer 2.  Host inverse-permutes and adds b2.
"""

import os

import numpy as np

try:
    import ml_dtypes

    BF16 = ml_dtypes.bfloat16
except ImportError:  # pragma: no cover
    BF16 = np.float32

# ---------------- geometry (hardcoded for nn_GAT_51694226374713) ------------
N_NODES = 50000
N_EDGES = 800000
N_CORES = 8
NB = 49                    # dst blocks per core
PB = 128                   # dst nodes (slots) per block
SLOTS = NB * PB            # 6272 node slots per core
V = N_CORES * SLOTS        # 50176 rows in the gathered node tables
SPLIT = (N_CORES // 2) * SLOTS   # table half boundary (row 25088)
CAP_HALF = 1280            # pass-2 per-(block, half) edge cap
F1 = 256                   # input features
H1, C1 = 8, 32             # layer-1 heads x channels
R1 = F1 + 2 * H1           # 272: layer-1 row payload  h | s | d
W1ROW = 384                # layer-1 row stride in bf16 elems (768 B)
NCLS = 40
R2 = NCLS + 2              # 42: layer-2 row payload
W2ROW = 128                # layer-2 row stride in bf16 elems (256 B)
NEG_SLOPE = 0.2
TAIL = 999.0               # dst-slot sentinel for pad edge slots

_CACHE: dict = {}


# ============================ host preprocessing ============================

def _greedy_pack(items, weights_list, caps_list, slot_caps):
    """Place items (ordered) into bins; weights_list/caps_list are parallel
    lists of per-item weight arrays and per-bin capacity arrays.  Returns
    bin_of_item.  Greedy: emptiest bin (by total weight) first, skipping
    bins where any cap or the slot cap would overflow."""
    import heapq

    n_bins = len(slot_caps)
    used = [np.zeros(n_bins, dtype=np.int64) for _ in weights_list]
    slots_used = np.zeros(n_bins, dtype=np.int64)
    total = np.zeros(n_bins, dtype=np.int64)
    bin_of = {}
    heap = [(0, b) for b in range(n_bins)]
    heapq.heapify(heap)
    for it in items:
        ws = [w[it] for w in weights_list]
        stash = []
        while True:
            if not heap:
                raise RuntimeError("packing failed; raise CAP_HALF")
            t, b = heapq.heappop(heap)
            if t != total[b]:
                continue  # stale
            if slots_used[b] >= slot_caps[b]:
                continue  # permanently full
            if any(
                used[k][b] + ws[k] > caps_list[k][b] for k in range(len(ws))
            ):
                stash.append((t, b))
                continue
            bin_of[it] = b
            slots_used[b] += 1
            for k in range(len(ws)):
                used[k][b] += ws[k]
            total[b] += sum(ws)
            heapq.heappush(heap, (int(total[b]), b))
            break
        for item in stash:
            heapq.heappush(heap, item)
    return bin_of


def _wrap_idx(lin):
    """Linear index array [n] (n % 16 == 0) -> dma_gather layout
    [128, n // 16] int16 (16-partition wrap, replicated to 128)."""
    n = lin.size
    assert n % 16 == 0
    w = lin.reshape(n // 16, 16).T.astype(np.int16)  # [16, n/16]
    return np.ascontiguousarray(np.tile(w, (8, 1)))  # [128, n/16]


def _ceil16(x):
    return (int(x) + 15) // 16 * 16


def _pack_graph(src, dst):
    """Assign nodes to (core, block, slot); route real edges (no self
    loops).  Returns perm_row [N] plus per-core device arrays and the
    baked per-block geometry (max over cores)."""
    deg = np.bincount(dst, minlength=N_NODES)

    # ---- pass 1: nodes -> cores, balancing total in-edges ----
    order = np.argsort(-deg, kind="stable")
    core_of = _greedy_pack(
        order,
        [deg],
        [np.full(N_CORES, 1 << 60, dtype=np.int64)],
        np.full(N_CORES, SLOTS, dtype=np.int64),
    )
    node_core = np.empty(N_NODES, dtype=np.int64)
    for nd, c in core_of.items():
        node_core[nd] = c

    half_b_src = node_core[src] >= (N_CORES // 2)
    degA = np.bincount(dst[~half_b_src], minlength=N_NODES)
    degB = np.bincount(dst[half_b_src], minlength=N_NODES)

    # ---- pass 2: per core, nodes -> blocks with per-half edge caps ----
    node_bin = np.empty(N_NODES, dtype=np.int64)
    node_slot_tmp = np.zeros(N_NODES, dtype=np.int64)
    for c in range(N_CORES):
        nodes_c = np.where(node_core == c)[0]
        ordc = nodes_c[np.argsort(-(deg[nodes_c]), kind="stable")]
        bin_of = _greedy_pack(
            ordc,
            [degA, degB],
            [
                np.full(NB, CAP_HALF, dtype=np.int64),
                np.full(NB, CAP_HALF, dtype=np.int64),
            ],
            np.full(NB, PB, dtype=np.int64),
        )
        # relabel bins by descending edge count so block b has similar
        # size on every core (per-block counts are baked as cross-core
        # maxima; aligned quantiles keep the padding small)
        btot = np.zeros(NB, dtype=np.int64)
        for nd in ordc:
            btot[bin_of[nd]] += deg[nd]
        rank = np.empty(NB, dtype=np.int64)
        rank[np.argsort(-btot, kind="stable")] = np.arange(NB)
        slots_used = np.zeros(NB, dtype=np.int64)
        for nd in ordc:
            b = rank[bin_of[nd]]
            node_bin[nd] = c * NB + b
            # slot assignment within block: arrival order
            node_slot_tmp[nd] = slots_used[b]
            slots_used[b] += 1

    perm_row = (node_bin * PB + node_slot_tmp).astype(np.int64)

    # ---- edge routing: per (bin, half), sorted by src row ----
    n_bins = N_CORES * NB
    ebin = node_bin[dst]
    src_row_e = perm_row[src]
    dst_slot_e = perm_row[dst] % PB
    keyhalf = half_b_src.astype(np.int64)
    sort_idx = np.lexsort((src_row_e, keyhalf, ebin))
    ebin_s = ebin[sort_idx]
    half_s = keyhalf[sort_idx]
    src_s = src_row_e[sort_idx]
    dsl_s = dst_slot_e[sort_idx]

    grp = ebin_s * 2 + half_s
    counts = np.bincount(grp, minlength=n_bins * 2)
    realA = counts[0::2].reshape(N_CORES, NB)
    realB = counts[1::2].reshape(N_CORES, NB)
    assert realA.max() <= CAP_HALF and realB.max() <= CAP_HALF

    # baked per-block geometry: max over cores, ceil to 16
    NAb = np.array([_ceil16(realA[:, b].max()) for b in range(NB)])
    NBb = np.array([_ceil16(realB[:, b].max()) for b in range(NB)])
    NAb = np.maximum(NAb, 16)
    NBb = np.maximum(NBb, 16)
    TbA = (NAb + PB - 1) // PB
    TbB = (NBb + PB - 1) // PB
    Tb = TbA + TbB
    TAmax = int(TbA.max())
    TBmax = int(TbB.max())
    Tmax = int(Tb.max())

    starts = np.zeros(n_bins * 2 + 1, dtype=np.int64)
    np.cumsum(counts, out=starts[1:])
    pos = np.arange(ebin_s.size) - starts[grp]

    b_of_bin = np.arange(n_bins) % NB
    # linear edge slot j within the block: A at [0, NAb), B at TbA*128 +
    j = np.where(half_s == 0, pos, (TbA[b_of_bin] * PB)[ebin_s] + pos)

    linA = np.zeros((n_bins, TAmax * PB), dtype=np.int64)
    linB = np.zeros((n_bins, TBmax * PB), dtype=np.int64)
    dlocF = np.full((n_bins, Tmax * PB), TAIL, dtype=np.float32)

    mA = half_s == 0
    linA[ebin_s[mA], pos[mA]] = src_s[mA]
    linB[ebin_s[~mA], pos[~mA]] = src_s[~mA] - SPLIT
    dlocF[ebin_s, j] = dsl_s

    idxA = np.zeros((N_CORES, NB, PB, TAmax * 8), dtype=np.int16)
    idxB = np.zeros((N_CORES, NB, PB, TBmax * 8), dtype=np.int16)
    for bi in range(n_bins):
        c, b = bi // NB, bi % NB
        wa = _wrap_idx(linA[bi, : NAb[b]])
        idxA[c, b, :, : wa.shape[1]] = wa
        wb = _wrap_idx(linB[bi, : NBb[b]])
        idxB[c, b, :, : wb.shape[1]] = wb

    # dloc [c, b, p, t] = dlocF[c, b, t*128 + p]
    dloc = np.ascontiguousarray(
        dlocF.reshape(N_CORES, NB, Tmax, PB).transpose(0, 1, 3, 2)
    ).astype(np.float32)
    dlocF = dlocF.reshape(N_CORES, NB, Tmax * PB).astype(BF16)

    geom = {
        "NAb": tuple(int(x) for x in NAb),
        "NBb": tuple(int(x) for x in NBb),
        "TbA": tuple(int(x) for x in TbA),
        "TbB": tuple(int(x) for x in TbB),
        "Tb": tuple(int(x) for x in Tb),
        "TAmax": TAmax,
        "TBmax": TBmax,
        "Tmax": Tmax,
    }
    return perm_row, idxA, idxB, dloc, dlocF, geom


def _expand_heads(a):
    """[H, C] attention vector -> block-diagonal [H*C, H] matrix."""
    h, c = a.shape
    m = np.zeros((h * c, h), dtype=np.float32)
    for i in range(h):
        m[i * c:(i + 1) * c, i] = a[i]
    return m


# ============================ device program ================================

def _build_program(geom):
    import concourse.bacc as bacc
    import concourse.mybir as mybir
    import concourse.tile as tile

    f32 = mybir.dt.float32
    bf16 = mybir.dt.bfloat16
    i16 = mybir.dt.int16
    Alu = mybir.AluOpType
    Act = mybir.ActivationFunctionType

    NAb, NBb = geom["NAb"], geom["NBb"]
    TbA, TbB, Tb = geom["TbA"], geom["TbB"], geom["Tb"]
    TAmax, TBmax, Tmax = geom["TAmax"], geom["TBmax"], geom["Tmax"]
    single_packet = bool(int(os.environ.get("GAT_SP", "0")))

    nc = bacc.Bacc(
        "TRN2", target_bir_lowering=False, debug=False, num_devices=N_CORES
    )

    # ---- kernel I/O ----
    xT = nc.dram_tensor("xT", [F1, SLOTS], f32, kind="ExternalInput")
    w1cat = nc.dram_tensor("w1cat", [F1, R1], f32, kind="ExternalInput")
    w2cat = nc.dram_tensor("w2cat", [F1, R2], bf16, kind="ExternalInput")
    iota_in = nc.dram_tensor("iota_row", [PB, PB], f32, kind="ExternalInput")
    iotaP_in = nc.dram_tensor("iota_col", [PB, PB], f32, kind="ExternalInput")
    ident_in = nc.dram_tensor("ident", [PB, PB], bf16, kind="ExternalInput")
    idxA_in = nc.dram_tensor(
        "idxA", [NB, PB, TAmax * 8], i16, kind="ExternalInput"
    )
    idxB_in = nc.dram_tensor(
        "idxB", [NB, PB, TBmax * 8], i16, kind="ExternalInput"
    )
    dloc_in = nc.dram_tensor("dloc", [NB, PB, Tmax], f32, kind="ExternalInput")
    dlocF_in = nc.dram_tensor(
        "dlocF", [NB, Tmax * PB], bf16, kind="ExternalInput"
    )
    out_dev = nc.dram_tensor("out_dev", [SLOTS, NCLS], f32, kind="ExternalOutput")

    # ---- internal DRAM ----
    hcat1_own = nc.dram_tensor("hcat1_own", [SLOTS, W1ROW], bf16, kind="Internal")
    hcat1_all = nc.dram_tensor(
        "hcat1_all", [V, W1ROW], bf16, kind="Internal", addr_space="Shared"
    )
    hcat2_own = nc.dram_tensor("hcat2_own", [SLOTS, W2ROW], bf16, kind="Internal")
    hcat2_all = nc.dram_tensor(
        "hcat2_all", [V, W2ROW], bf16, kind="Internal", addr_space="Shared"
    )

    groups = [list(range(N_CORES))]

    with tile.TileContext(nc) as tc:
        with (
            tc.tile_pool(name="persist", bufs=1) as pp,
            tc.tile_pool(name="sb", bufs=2) as sb,
            tc.tile_pool(name="psA", bufs=2, space="PSUM") as psA,
            tc.tile_pool(name="psB", bufs=2, space="PSUM") as psB,
        ):
            # ---------------- persistent tiles ----------------
            iota_sb = pp.tile([PB, PB], f32, tag="iota")
            nc.sync.dma_start(out=iota_sb[:], in_=iota_in[:, :])
            iotaP_sb = pp.tile([PB, PB], f32, tag="iotaP")
            nc.sync.dma_start(out=iotaP_sb[:], in_=iotaP_in[:, :])
            ident_sb = pp.tile([PB, PB], bf16, tag="ident")
            nc.sync.dma_start(out=ident_sb[:], in_=ident_in[:, :])
            ones_sb = pp.tile([1, PB], bf16, tag="ones")
            nc.vector.memset(ones_sb[:], 1.0)

            w1_sb = [
                pp.tile([PB, R1], f32, tag=f"w1_{k}", name=f"w1_sb{k}")
                for k in range(2)
            ]
            for k in range(2):
                nc.sync.dma_start(out=w1_sb[k][:], in_=w1cat[k * PB:(k + 1) * PB, :])
            w2_sb = [
                pp.tile([PB, R2], bf16, tag=f"w2_{k}", name=f"w2_sb{k}")
                for k in range(2)
            ]
            for k in range(2):
                nc.sync.dma_start(out=w2_sb[k][:], in_=w2cat[k * PB:(k + 1) * PB, :])

            xT_sb = [
                pp.tile([PB, SLOTS], f32, tag=f"xT{k}", name=f"xT_sb{k}")
                for k in range(2)
            ]
            for k in range(2):
                nc.sync.dma_start(out=xT_sb[k][:], in_=xT[k * PB:(k + 1) * PB, :])
            h2T_sb = [
                pp.tile([PB, SLOTS], bf16, tag=f"h2T{k}", name=f"h2T_sb{k}")
                for k in range(2)
            ]

            def edge_layer(layer):
                """Per-block edge pass; layer is 1 or 2."""
                if layer == 1:
                    table_all, table_own = hcat1_all, hcat1_own
                    WROW, RP, NF, NH = W1ROW, R1, F1, H1
                else:
                    table_all, table_own = hcat2_all, hcat2_own
                    WROW, RP, NF, NH = W2ROW, R2, NCLS, 1
                sfx = f"L{layer}"
                for b in range(NB):
                    tba, tbb, tb = TbA[b], TbB[b], Tb[b]
                    na, nb_ = NAb[b], NBb[b]
                    iA = sb.tile([PB, TAmax * 8], i16, tag="iA")
                    nc.sync.dma_start(out=iA[:], in_=idxA_in[b, :, :])
                    iB = sb.tile([PB, TBmax * 8], i16, tag="iB")
                    nc.sync.dma_start(out=iB[:], in_=idxB_in[b, :, :])
                    dl = sb.tile([PB, Tmax], f32, tag="dl")
                    nc.sync.dma_start(out=dl[:], in_=dloc_in[b, :, :])
                    dlF = sb.tile([1, Tmax * PB], bf16, tag="dlF")
                    nc.sync.dma_start(out=dlF[:], in_=dlocF_in[b:b + 1, :])
                    own = sb.tile([PB, RP], bf16, tag="own" + sfx)
                    nc.sync.dma_start(
                        out=own[:], in_=table_own[b * PB:(b + 1) * PB, 0:RP]
                    )

                    G = sb.tile([PB, Tmax * WROW], bf16, tag="G" + sfx)
                    G3 = G[:].rearrange("p (t f) -> p t f", t=Tmax)
                    # zero the partial tail tiles (junk killed by zero
                    # one-hot columns, but must stay finite)
                    nc.vector.memset(G3[:, tba - 1, :], 0.0)
                    nc.vector.memset(G3[:, tb - 1, :], 0.0)
                    nc.gpsimd.dma_gather(
                        out_ap=G3[:, 0:tba, :],
                        in_ap=table_all[0:SPLIT, :],
                        idxs_ap=iA[:][:, 0:na // 16],
                        num_idxs=na,
                        num_idxs_reg=na,
                        elem_size=WROW,
                        single_packet=single_packet,
                    )
                    nc.gpsimd.dma_gather(
                        out_ap=G3[:, tba:tb, :],
                        in_ap=table_all[SPLIT:V, :],
                        idxs_ap=iB[:][:, 0:nb_ // 16],
                        num_idxs=nb_,
                        num_idxs_reg=nb_,
                        elem_size=WROW,
                        single_packet=single_packet,
                    )

                    # one-hot Bm[e, (t, d)] for aggregation
                    Bm = sb.tile([PB, Tmax * PB], bf16, tag="Bm")
                    Bm3 = Bm[:].rearrange("p (t d) -> p t d", t=Tmax)
                    nc.vector.tensor_tensor(
                        out=Bm3[:, 0:tb, :],
                        in0=dl[:][:, 0:tb, None].broadcast_to([PB, tb, PB]),
                        in1=iota_sb[:][:, None, :].broadcast_to([PB, tb, PB]),
                        op=Alu.is_equal,
                    )

                    # alpha_dst per edge slot: replicate dst-slot ids down
                    # the partitions (1-row matmul), is_equal against the
                    # column-iota, then BmT.T @ dblk
                    D2 = psB.tile([PB, Tmax * NH], f32, tag="D2")
                    for t in range(tb):
                        Mt = psB.tile([PB, PB], f32, tag="tmp")
                        nc.tensor.matmul(
                            out=Mt[:],
                            lhsT=ones_sb[:],
                            rhs=dlF[:][:, t * PB:(t + 1) * PB],
                            start=True,
                            stop=True,
                        )
                        BmT = sb.tile([PB, PB], bf16, tag="BmT")
                        nc.vector.tensor_tensor(
                            out=BmT[:], in0=Mt[:], in1=iotaP_sb[:],
                            op=Alu.is_equal,
                        )
                        nc.tensor.matmul(
                            out=D2[:][:, t * NH:(t + 1) * NH],
                            lhsT=BmT[:],
                            rhs=own[:][:, NF + NH:NF + 2 * NH],
                            start=True,
                            stop=True,
                        )

                    # logits -> p = exp(leaky_relu(s_src + d_dst))
                    sf = sb.tile([PB, Tmax * NH], f32, tag="sf")
                    sf3 = sf[:].rearrange("p (t h) -> p t h", t=Tmax)
                    nc.vector.tensor_copy(
                        out=sf3[:, 0:tb, :], in_=G3[:, 0:tb, NF:NF + NH]
                    )
                    lg = sb.tile([PB, Tmax * NH], f32, tag="lg")
                    nc.vector.tensor_tensor(
                        out=lg[:][:, 0:tb * NH],
                        in0=sf[:][:, 0:tb * NH],
                        in1=D2[:][:, 0:tb * NH],
                        op=Alu.add,
                    )
                    lg2 = sb.tile([PB, Tmax * NH], f32, tag="lg2")
                    nc.vector.tensor_scalar_mul(
                        out=lg2[:][:, 0:tb * NH], in0=lg[:][:, 0:tb * NH],
                        scalar1=NEG_SLOPE,
                    )
                    nc.vector.tensor_tensor(
                        out=lg[:][:, 0:tb * NH],
                        in0=lg[:][:, 0:tb * NH],
                        in1=lg2[:][:, 0:tb * NH],
                        op=Alu.max,
                    )
                    p = sb.tile([PB, Tmax * NH], bf16, tag="p")
                    nc.scalar.activation(
                        out=p[:][:, 0:tb * NH], in_=lg[:][:, 0:tb * NH],
                        func=Act.Exp,
                    )
                    p3 = p[:].rearrange("p (t h) -> p t h", t=Tmax)

                    # in-place: G[:, :, 0:NF] *= p ; G[:, :, NF:NF+NH] = p
                    out4 = G3[:, 0:tb, 0:NF].rearrange(
                        "p t (h c) -> p t h c", h=NH
                    )
                    nc.vector.tensor_tensor(
                        out=out4,
                        in0=out4,
                        in1=p3[:, 0:tb, :, None].broadcast_to(
                            [PB, tb, NH, NF // NH]
                        ),
                        op=Alu.mult,
                    )
                    nc.vector.tensor_copy(
                        out=G3[:, 0:tb, NF:NF + NH], in_=p3[:, 0:tb, :]
                    )

                    # accumulate out[d] = B.T @ [p*h | p] over edge tiles
                    po = psA.tile([PB, R1], f32, tag="mm")
                    for t in range(tb):
                        nc.tensor.matmul(
                            out=po[:][:, 0:NF + NH],
                            lhsT=Bm[:][:, t * PB:(t + 1) * PB],
                            rhs=G3[:, t, 0:NF + NH],
                            start=(t == 0),
                            stop=(t == tb - 1),
                        )

                    # self loop: p_self = exp(leaky_relu(s_own + d_own))
                    sd = sb.tile([PB, NH], f32, tag="sd")
                    nc.vector.tensor_tensor(
                        out=sd[:], in0=own[:][:, NF:NF + NH],
                        in1=own[:][:, NF + NH:NF + 2 * NH], op=Alu.add,
                    )
                    sd2 = sb.tile([PB, NH], f32, tag="sd2")
                    nc.vector.tensor_scalar_mul(
                        out=sd2[:], in0=sd[:], scalar1=NEG_SLOPE
                    )
                    nc.vector.tensor_tensor(
                        out=sd[:], in0=sd[:], in1=sd2[:], op=Alu.max
                    )
                    pself = sb.tile([PB, NH], f32, tag="pself")
                    nc.scalar.activation(out=pself[:], in_=sd[:], func=Act.Exp)
                    pselfb = sb.tile([PB, NH], bf16, tag="pselfb")
                    nc.vector.tensor_copy(out=pselfb[:], in_=pself[:])

                    of = sb.tile([PB, NF + NH], f32, tag="of" + sfx)
                    nc.scalar.copy(out=of[:], in_=po[:][:, 0:NF + NH])
                    slh = sb.tile([PB, NF], f32, tag="slh" + sfx)
                    slh3 = slh[:].rearrange("p (h c) -> p h c", h=NH)
                    nc.vector.tensor_tensor(
                        out=slh3,
                        in0=own[:][:, 0:NF].rearrange("p (h c) -> p h c", h=NH),
                        in1=pselfb[:][:, :, None].broadcast_to(
                            [PB, NH, NF // NH]
                        ),
                        op=Alu.mult,
                    )
                    nc.vector.tensor_tensor(
                        out=of[:][:, 0:NF], in0=of[:][:, 0:NF], in1=slh[:],
                        op=Alu.add,
                    )
                    nc.vector.tensor_tensor(
                        out=of[:][:, NF:NF + NH], in0=of[:][:, NF:NF + NH],
                        in1=pself[:], op=Alu.add,
                    )
                    rden = sb.tile([PB, NH], f32, tag="rden")
                    nc.vector.reciprocal(out=rden[:], in_=of[:][:, NF:NF + NH])

                    o1 = sb.tile([PB, NF], f32, tag="o1" + sfx)
                    o13 = o1[:].rearrange("p (h c) -> p h c", h=NH)
                    nc.vector.tensor_tensor(
                        out=o13,
                        in0=of[:][:, 0:NF].rearrange("p (h c) -> p h c", h=NH),
                        in1=rden[:][:, :, None].broadcast_to(
                            [PB, NH, NF // NH]
                        ),
                        op=Alu.mult,
                    )

                    if layer == 1:
                        # elu(x) = max(x,0) - 1 + exp(min(x,0)), -> h2 bf16
                        mneg = sb.tile([PB, NF], f32, tag="mneg")
                        nc.vector.tensor_scalar_min(
                            out=mneg[:], in0=o1[:], scalar1=0.0
                        )
                        eneg = sb.tile([PB, NF], f32, tag="eneg")
                        nc.scalar.activation(
                            out=eneg[:], in_=mneg[:], func=Act.Exp
                        )
                        h2a = sb.tile([PB, NF], f32, tag="h2a")
                        nc.vector.tensor_scalar(
                            out=h2a[:], in0=o1[:], scalar1=0.0, scalar2=-1.0,
                            op0=Alu.max, op1=Alu.add,
                        )
                        h2 = sb.tile([PB, NF], bf16, tag="h2")
                        nc.vector.tensor_tensor(
                            out=h2[:], in0=h2a[:], in1=eneg[:], op=Alu.add
                        )
                        for k in range(2):
                            pt = psB.tile([PB, PB], bf16, tag="tr")
                            nc.tensor.transpose(
                                out=pt[:],
                                in_=h2[:][:, k * PB:(k + 1) * PB],
                                identity=ident_sb[:],
                            )
                            nc.scalar.copy(
                                out=h2T_sb[k][:][:, b * PB:(b + 1) * PB],
                                in_=pt[:],
                            )
                    else:
                        nc.sync.dma_start(
                            out=out_dev[b * PB:(b + 1) * PB, :],
                            in_=o1[:][:, 0:NCLS],
                        )

            with nc.named_scope("gat"):
                # ---------------- phase A: hcat1 = x @ W1cat ----------------
                for nb_i in range(NB):
                    ps = psA.tile([PB, R1], f32, tag="mm")
                    for k in range(2):
                        nc.tensor.matmul(
                            out=ps[:],
                            lhsT=xT_sb[k][:][:, nb_i * PB:(nb_i + 1) * PB],
                            rhs=w1_sb[k][:],
                            start=(k == 0),
                            stop=(k == 1),
                        )
                    hc = sb.tile([PB, R1], bf16, tag="hc1")
                    nc.scalar.copy(out=hc[:], in_=ps[:])
                    nc.sync.dma_start(
                        out=hcat1_own[nb_i * PB:(nb_i + 1) * PB, 0:R1],
                        in_=hc[:],
                    )

                import concourse.mybir as _mb

                nc.gpsimd.collective_compute(
                    "AllGather",
                    _mb.AluOpType.bypass,
                    replica_groups=groups,
                    ins=[hcat1_own[:, :].opt()],
                    outs=[hcat1_all[:, :].opt()],
                )

                edge_layer(1)

                # ---------------- phase C: hcat2 = h2 @ W2cat ---------------
                for nb_i in range(NB):
                    ps = psA.tile([PB, R1], f32, tag="mm")
                    for k in range(2):
                        nc.tensor.matmul(
                            out=ps[:][:, 0:R2],
                            lhsT=h2T_sb[k][:][:, nb_i * PB:(nb_i + 1) * PB],
                            rhs=w2_sb[k][:],
                            start=(k == 0),
                            stop=(k == 1),
                        )
                    hc2 = sb.tile([PB, R2], bf16, tag="hc2")
                    nc.scalar.copy(out=hc2[:], in_=ps[:][:, 0:R2])
                    nc.sync.dma_start(
                        out=hcat2_own[nb_i * PB:(nb_i + 1) * PB, 0:R2],
                        in_=hc2[:],
                    )

                nc.gpsimd.collective_compute(
                    "AllGather",
                    _mb.AluOpType.bypass,
                    replica_groups=groups,
                    ins=[hcat2_own[:, :].opt()],
                    outs=[hcat2_all[:, :].opt()],
                )

                edge_layer(2)

    nc.compile()
    return nc


# ============================ top-level entry ===============================

def _prepare(inputs):
    x = np.ascontiguousarray(np.asarray(inputs["x"], dtype=np.float32))
    edge_index = np.asarray(inputs["edge_index"], dtype=np.int64)
    w1 = np.asarray(inputs["w1"], dtype=np.float32)
    a_src1 = np.asarray(inputs["a_src1"], dtype=np.float32)
    a_dst1 = np.asarray(inputs["a_dst1"], dtype=np.float32)
    b1 = np.asarray(inputs["b1"], dtype=np.float32)
    w2 = np.asarray(inputs["w2"], dtype=np.float32)
    a_src2 = np.asarray(inputs["a_src2"], dtype=np.float32)
    a_dst2 = np.asarray(inputs["a_dst2"], dtype=np.float32)
    b2 = np.asarray(inputs["b2"], dtype=np.float32)

    assert x.shape == (N_NODES, F1) and edge_index.shape == (2, N_EDGES)
    assert np.all(np.abs(b1) == 0.0), "kernel hardcodes b1 == 0"

    src = edge_index[0]
    dst = edge_index[1]
    perm_row, idxA, idxB, dloc, dlocF, geom = _pack_graph(src, dst)

    w1cat = np.concatenate(
        [w1, w1 @ _expand_heads(a_src1), w1 @ _expand_heads(a_dst1)], axis=1
    ).astype(np.float32)
    w2cat = np.concatenate(
        [w2, w2 @ _expand_heads(a_src2), w2 @ _expand_heads(a_dst2)], axis=1
    ).astype(BF16)

    xp = np.zeros((V, F1), dtype=np.float32)
    xp[perm_row] = x
    iota_row = np.broadcast_to(np.arange(PB, dtype=np.float32), (PB, PB)).copy()
    iota_col = np.ascontiguousarray(iota_row.T)
    ident = np.eye(PB, dtype=np.float32).astype(BF16)

    in_maps = []
    for c in range(N_CORES):
        xT_c = np.ascontiguousarray(xp[c * SLOTS:(c + 1) * SLOTS].T)
        in_maps.append(
            {
                "xT": xT_c,
                "w1cat": w1cat,
                "w2cat": w2cat,
                "iota_row": iota_row,
                "iota_col": iota_col,
                "ident": ident,
                "idxA": idxA[c],
                "idxB": idxB[c],
                "dloc": dloc[c],
                "dlocF": dlocF[c],
            }
        )
    return in_maps, perm_row, b2, geom


def _assemble(core_outs, perm_row, b2):
    out_all = np.concatenate(core_outs, axis=0)
    out = out_all[perm_row] + b2[None, :]
    return out.astype(np.float32)


def kernel(**inputs) -> np.ndarray:
    in_maps, perm_row, b2, geom = _prepare(inputs)

    import concourse.bass_utils as bass_utils

    key = ("nc", tuple(sorted(geom.items())))
    if key not in _CACHE:
        _CACHE.clear()
        _CACHE[key] = _build_program(geom)
    nc = _CACHE[key]

    trace = bool(int(os.environ.get("GAT_TRACE", "0")))
    res = bass_utils.run_bass_kernel_spmd(
        nc,
        in_maps,
        core_ids=list(range(N_CORES)),
        trace=trace,
        trace_cores=list(range(N_CORES)) if trace else None,
        stitch_traces=trace,
    )
    _CACHE["last_results"] = res

    return _assemble([r["out_dev"] for r in res.results], perm_row, b2)


# revision 16
# speedup vs baseline: 3.2197x; 1.1125x over previous
"""Two-layer GAT (PyG semantics) on 8 Trainium2 NeuronCores.

Gather-lean bf16 redesign.  The baseline spent 4.2 of 5.1 ms on GPSIMD Q7
descriptor generation for dma_gather (~8 ns per gathered row, 502k rows
per core).  This version cuts gather indices ~2.5x and moves everything
else off the critical engine:

  * No self-loop edges in the edge list: the self-loop term p_self*h[own]
    is computed per dst block from the locally-stored rows (every node
    slot gets exactly one self loop, including empty slots, which hold
    h=0 and stay harmless).
  * No per-edge dst-alpha gather (the baseline's Dg/D2, 50% of indices).
    Instead alpha_dst is broadcast per block on the PE: a one-partition
    matmul replicates the per-edge dst-slot ids down 128 partitions, DVE
    is_equal against a column-iota builds the transposed one-hot
    BmT[d, e], and BmT.T @ dblk gives alpha_dst per edge slot.
  * bf16 node tables: layer-1 rows [h(256)|s(8)|d(8)] at a 768 B stride
    (vs 1280 B f32), layer-2 rows [h2(40)|s|d] at 256 B.  Gathers move
    real edges only (per-block counts baked at trace time as the max
    across cores; tails are memset + killed by zero one-hot columns).
  * bf16 PE matmuls (1 cycle/row vs 4 for fp32) for edge aggregation
    B.T @ [p*h | p], with f32 PSUM accumulation.

Host: add no self loops; assign nodes to 8 cores balancing in-edges,
pack each core's nodes into 49 blocks of 128 slots; route edges to the
block owning their dst, split by src table half (int16 gather indices
address <=32768 rows; the table splits at row 25088), sorted by src row.
Device: hcat = x @ [W|W@a_src|W@a_dst] per core, AllGather to the full
table, per-block gather + softmax + one-hot-matmul aggregation + ELU,
then the same again for layer 2.  Host inverse-permutes and adds b2.
"""

import os

import numpy as np

try:
    import ml_dtypes

    BF16 = ml_dtypes.bfloat16
except ImportError:  # pragma: no cover
    BF16 = np.float32

# ---------------- geometry (hardcoded for nn_GAT_51694226374713) ------------
N_NODES = 50000
N_EDGES = 800000
N_CORES = 8
NB = 49                    # dst blocks per core
PB = 128                   # dst nodes (slots) per block
SLOTS = NB * PB            # 6272 node slots per core
V = N_CORES * SLOTS        # 50176 rows in the gathered node tables
SPLIT = (N_CORES // 2) * SLOTS   # table half boundary (row 25088)
CAP_HALF = 1280            # pass-2 per-(block, half) edge cap
F1 = 256                   # input features
H1, C1 = 8, 32             # layer-1 heads x channels
R1 = F1 + 2 * H1           # 272: layer-1 row payload  h | s | d
W1ROW = 384                # layer-1 row stride in bf16 elems (768 B)
NCLS = 40
R2 = NCLS + 2              # 42: layer-2 row payload
W2ROW = 128                # layer-2 row stride in bf16 elems (256 B)
NEG_SLOPE = 0.2
TAIL = 999.0               # dst-slot sentinel for pad edge slots

_CACHE: dict = {}


# ============================ host preprocessing ============================

def _greedy_pack(items, weights_list, caps_list, slot_caps):
    """Place items (ordered) into bins; weights_list/caps_list are parallel
    lists of per-item weight arrays and per-bin capacity arrays.  Returns
    bin_of_item.  Greedy: emptiest bin (by total weight) first, skipping
    bins where any cap or the slot cap would overflow."""
    import heapq

    n_bins = len(slot_caps)
    used = [np.zeros(n_bins, dtype=np.int64) for _ in weights_list]
    slots_used = np.zeros(n_bins, dtype=np.int64)
    total = np.zeros(n_bins, dtype=np.int64)
    bin_of = {}
    heap = [(0, b) for b in range(n_bins)]
    heapq.heapify(heap)
    for it in items:
        ws = [w[it] for w in weights_list]
        stash = []
        while True:
            if not heap:
                raise RuntimeError("packing failed; raise CAP_HALF")
            t, b = heapq.heappop(heap)
            if t != total[b]:
                continue  # stale
            if slots_used[b] >= slot_caps[b]:
                continue  # permanently full
            if any(
                used[k][b] + ws[k] > caps_list[k][b] for k in range(len(ws))
            ):
                stash.append((t, b))
                continue
            bin_of[it] = b
            slots_used[b] += 1
            for k in range(len(ws)):
                used[k][b] += ws[k]
            total[b] += sum(ws)
            heapq.heappush(heap, (int(total[b]), b))
            break
        for item in stash:
            heapq.heappush(heap, item)
    return bin_of


def _wrap_idx(lin):
    """Linear index array [n] (n % 16 == 0) -> dma_gather layout
    [128, n // 16] int16 (16-partition wrap, replicated to 128)."""
    n = lin.size
    assert n % 16 == 0
    w = lin.reshape(n // 16, 16).T.astype(np.int16)  # [16, n/16]
    return np.ascontiguousarray(np.tile(w, (8, 1)))  # [128, n/16]


def _ceil16(x):
    return (int(x) + 15) // 16 * 16


def _pack_graph(src, dst):
    """Assign nodes to (core, block, slot); route real edges (no self
    loops).  Returns perm_row [N] plus per-core device arrays and the
    baked per-block geometry (max over cores)."""
    deg = np.bincount(dst, minlength=N_NODES)

    # ---- pass 1: nodes -> cores, balancing total in-edges ----
    order = np.argsort(-deg, kind="stable")
    core_of = _greedy_pack(
        order,
        [deg],
        [np.full(N_CORES, 1 << 60, dtype=np.int64)],
        np.full(N_CORES, SLOTS, dtype=np.int64),
    )
    node_core = np.empty(N_NODES, dtype=np.int64)
    for nd, c in core_of.items():
        node_core[nd] = c

    half_b_src = node_core[src] >= (N_CORES // 2)
    degA = np.bincount(dst[~half_b_src], minlength=N_NODES)
    degB = np.bincount(dst[half_b_src], minlength=N_NODES)

    # ---- pass 2: per core, nodes -> blocks with per-half edge caps ----
    node_bin = np.empty(N_NODES, dtype=np.int64)
    node_slot_tmp = np.zeros(N_NODES, dtype=np.int64)
    for c in range(N_CORES):
        nodes_c = np.where(node_core == c)[0]
        ordc = nodes_c[np.argsort(-(deg[nodes_c]), kind="stable")]
        bin_of = _greedy_pack(
            ordc,
            [degA, degB],
            [
                np.full(NB, CAP_HALF, dtype=np.int64),
                np.full(NB, CAP_HALF, dtype=np.int64),
            ],
            np.full(NB, PB, dtype=np.int64),
        )
        # relabel bins by descending edge count so block b has similar
        # size on every core (per-block counts are baked as cross-core
        # maxima; aligned quantiles keep the padding small)
        btot = np.zeros(NB, dtype=np.int64)
        for nd in ordc:
            btot[bin_of[nd]] += deg[nd]
        rank = np.empty(NB, dtype=np.int64)
        rank[np.argsort(-btot, kind="stable")] = np.arange(NB)
        slots_used = np.zeros(NB, dtype=np.int64)
        for nd in ordc:
            b = rank[bin_of[nd]]
            node_bin[nd] = c * NB + b
            # slot assignment within block: arrival order
            node_slot_tmp[nd] = slots_used[b]
            slots_used[b] += 1

    perm_row = (node_bin * PB + node_slot_tmp).astype(np.int64)

    # ---- edge routing: per (bin, half), sorted by src row ----
    n_bins = N_CORES * NB
    ebin = node_bin[dst]
    src_row_e = perm_row[src]
    dst_slot_e = perm_row[dst] % PB
    keyhalf = half_b_src.astype(np.int64)
    sort_idx = np.lexsort((src_row_e, keyhalf, ebin))
    ebin_s = ebin[sort_idx]
    half_s = keyhalf[sort_idx]
    src_s = src_row_e[sort_idx]
    dsl_s = dst_slot_e[sort_idx]

    grp = ebin_s * 2 + half_s
    counts = np.bincount(grp, minlength=n_bins * 2)
    realA = counts[0::2].reshape(N_CORES, NB)
    realB = counts[1::2].reshape(N_CORES, NB)
    assert realA.max() <= CAP_HALF and realB.max() <= CAP_HALF

    # baked per-block geometry: max over cores, ceil to full 128-tiles so
    # paired gathers stay tile-aligned
    NAb = np.array([-(-max(int(realA[:, b].max()), 1) // PB) * PB for b in range(NB)])
    NBb = np.array([-(-max(int(realB[:, b].max()), 1) // PB) * PB for b in range(NB)])
    TbA = NAb // PB
    TbB = NBb // PB
    Tb = TbA + TbB
    TAmax = int(TbA.max())
    TBmax = int(TbB.max())
    Tmax = int(Tb.max())
    Nb2 = [list(range(i, min(i + 2, NB))) for i in range(0, NB, 2)]

    starts = np.zeros(n_bins * 2 + 1, dtype=np.int64)
    np.cumsum(counts, out=starts[1:])
    pos = np.arange(ebin_s.size) - starts[grp]

    b_of_bin = np.arange(n_bins) % NB
    # linear edge slot j within the block: A at [0, NAb), B at TbA*128 +
    j = np.where(half_s == 0, pos, (TbA[b_of_bin] * PB)[ebin_s] + pos)

    linA = np.zeros((n_bins, TAmax * PB), dtype=np.int64)
    linB = np.zeros((n_bins, TBmax * PB), dtype=np.int64)
    dlocF = np.full((n_bins, Tmax * PB), TAIL, dtype=np.float32)

    mA = half_s == 0
    linA[ebin_s[mA], pos[mA]] = src_s[mA]
    linB[ebin_s[~mA], pos[~mA]] = src_s[~mA] - SPLIT
    dlocF[ebin_s, j] = dsl_s

    # paired idx arrays: pair p = blocks NB2[p]; horizontal concat of the
    # 16-wrapped per-block index arrays (valid because counts are %16)
    NP = len(Nb2)
    WA2 = max(sum(NAb[b] for b in pr) for pr in Nb2) // 16
    WB2 = max(sum(NBb[b] for b in pr) for pr in Nb2) // 16
    idxA = np.zeros((N_CORES, NP, PB, WA2), dtype=np.int16)
    idxB = np.zeros((N_CORES, NP, PB, WB2), dtype=np.int16)
    for c in range(N_CORES):
        for pi, pr in enumerate(Nb2):
            wa = np.hstack([_wrap_idx(linA[c * NB + b, : NAb[b]]) for b in pr])
            idxA[c, pi, :, : wa.shape[1]] = wa
            wb = np.hstack([_wrap_idx(linB[c * NB + b, : NBb[b]]) for b in pr])
            idxB[c, pi, :, : wb.shape[1]] = wb

    # dloc [c, b, p, t] = dlocF[c, b, t*128 + p]
    dloc = np.ascontiguousarray(
        dlocF.reshape(N_CORES, NB, Tmax, PB).transpose(0, 1, 3, 2)
    ).astype(np.float32)
    dlocF = dlocF.reshape(N_CORES, NB, Tmax * PB).astype(BF16)

    geom = {
        "NAb": tuple(int(x) for x in NAb),
        "NBb": tuple(int(x) for x in NBb),
        "TbA": tuple(int(x) for x in TbA),
        "TbB": tuple(int(x) for x in TbB),
        "Tb": tuple(int(x) for x in Tb),
        "TAmax": TAmax,
        "TBmax": TBmax,
        "Tmax": Tmax,
        "WA2": WA2,
        "WB2": WB2,
    }
    return perm_row, idxA, idxB, dloc, dlocF, geom


def _expand_heads(a):
    """[H, C] attention vector -> block-diagonal [H*C, H] matrix."""
    h, c = a.shape
    m = np.zeros((h * c, h), dtype=np.float32)
    for i in range(h):
        m[i * c:(i + 1) * c, i] = a[i]
    return m


# ============================ device program ================================

def _build_program(geom):
    import concourse.bacc as bacc
    import concourse.mybir as mybir
    import concourse.tile as tile

    f32 = mybir.dt.float32
    bf16 = mybir.dt.bfloat16
    i16 = mybir.dt.int16
    Alu = mybir.AluOpType
    Act = mybir.ActivationFunctionType

    NAb, NBb = geom["NAb"], geom["NBb"]
    TbA, TbB, Tb = geom["TbA"], geom["TbB"], geom["Tb"]
    TAmax, TBmax, Tmax = geom["TAmax"], geom["TBmax"], geom["Tmax"]
    WA2, WB2 = geom["WA2"], geom["WB2"]
    PAIRS = [list(range(i, min(i + 2, NB))) for i in range(0, NB, 2)]
    PT = max(sum(Tb[b] for b in pr) for pr in PAIRS)  # G super-tile tiles
    single_packet = bool(int(os.environ.get("GAT_SP", "0")))
    MCH = 4                    # BmT build chunk; 4*128 = 512 = max mm free dim

    nc = bacc.Bacc(
        "TRN2", target_bir_lowering=False, debug=False, num_devices=N_CORES
    )

    # ---- kernel I/O ----
    xT = nc.dram_tensor("xT", [F1, SLOTS], bf16, kind="ExternalInput")
    w1cat = nc.dram_tensor("w1cat", [F1, R1], bf16, kind="ExternalInput")
    w2cat = nc.dram_tensor("w2cat", [F1, R2], bf16, kind="ExternalInput")
    iota_in = nc.dram_tensor("iota_row", [PB, PB], f32, kind="ExternalInput")
    iotaP_in = nc.dram_tensor("iota_col", [PB, PB], f32, kind="ExternalInput")
    ident_in = nc.dram_tensor("ident", [PB, PB], bf16, kind="ExternalInput")
    idxA_in = nc.dram_tensor(
        "idxA", [len(PAIRS), PB, WA2], i16, kind="ExternalInput"
    )
    idxB_in = nc.dram_tensor(
        "idxB", [len(PAIRS), PB, WB2], i16, kind="ExternalInput"
    )
    dloc_in = nc.dram_tensor("dloc", [NB, PB, Tmax], f32, kind="ExternalInput")
    dlocF_in = nc.dram_tensor(
        "dlocF", [NB, Tmax * PB], bf16, kind="ExternalInput"
    )
    out_dev = nc.dram_tensor("out_dev", [SLOTS, NCLS], f32, kind="ExternalOutput")

    # ---- internal DRAM ----
    hcat1_own = nc.dram_tensor("hcat1_own", [SLOTS, W1ROW], bf16, kind="Internal")
    hcat1_all = nc.dram_tensor(
        "hcat1_all", [V, W1ROW], bf16, kind="Internal", addr_space="Shared"
    )
    hcat2_own = nc.dram_tensor("hcat2_own", [SLOTS, W2ROW], bf16, kind="Internal")
    hcat2_all = nc.dram_tensor(
        "hcat2_all", [V, W2ROW], bf16, kind="Internal", addr_space="Shared"
    )

    groups = [list(range(N_CORES))]

    with tile.TileContext(nc) as tc:
        with (
            tc.tile_pool(name="persist", bufs=1) as pp,
            tc.tile_pool(name="sb", bufs=2) as sb,
            tc.tile_pool(name="psA", bufs=2, space="PSUM") as psA,
            tc.tile_pool(name="psB", bufs=2, space="PSUM") as psB,
            tc.tile_pool(name="psC", bufs=1, space="PSUM") as psC,
        ):
            # ---------------- persistent tiles ----------------
            iota_sb = pp.tile([PB, PB], f32, tag="iota")
            nc.sync.dma_start(out=iota_sb[:], in_=iota_in[:, :])
            iotaP_sb = pp.tile([PB, PB], f32, tag="iotaP")
            nc.sync.dma_start(out=iotaP_sb[:], in_=iotaP_in[:, :])
            ident_sb = pp.tile([PB, PB], bf16, tag="ident")
            nc.sync.dma_start(out=ident_sb[:], in_=ident_in[:, :])
            ones_sb = pp.tile([1, PB], bf16, tag="ones")
            nc.vector.memset(ones_sb[:], 1.0)

            w1_sb = [
                pp.tile([PB, R1], bf16, tag=f"w1_{k}", name=f"w1_sb{k}")
                for k in range(2)
            ]
            for k in range(2):
                nc.sync.dma_start(out=w1_sb[k][:], in_=w1cat[k * PB:(k + 1) * PB, :])
            w2_sb = [
                pp.tile([PB, R2], bf16, tag=f"w2_{k}", name=f"w2_sb{k}")
                for k in range(2)
            ]
            for k in range(2):
                nc.sync.dma_start(out=w2_sb[k][:], in_=w2cat[k * PB:(k + 1) * PB, :])

            xT_sb = [
                pp.tile([PB, SLOTS], bf16, tag=f"xT{k}", name=f"xT_sb{k}")
                for k in range(2)
            ]
            for k in range(2):
                nc.sync.dma_start(out=xT_sb[k][:], in_=xT[k * PB:(k + 1) * PB, :])
            h2T_sb = [
                pp.tile([PB, SLOTS], bf16, tag=f"h2T{k}", name=f"h2T_sb{k}")
                for k in range(2)
            ]

            def phase_c_block(nb_i):
                ps = psA.tile([PB, R1], f32, tag="mm")
                for k in range(2):
                    nc.tensor.matmul(
                        out=ps[:][:, 0:R2],
                        lhsT=h2T_sb[k][:][:, nb_i * PB:(nb_i + 1) * PB],
                        rhs=w2_sb[k][:],
                        start=(k == 0),
                        stop=(k == 1),
                    )
                hc2 = sb.tile([PB, R2], bf16, tag="hc2")
                nc.scalar.copy(out=hc2[:], in_=ps[:][:, 0:R2])
                nc.sync.dma_start(
                    out=hcat2_own[nb_i * PB:(nb_i + 1) * PB, 0:R2],
                    in_=hc2[:],
                )

            def edge_block(layer, b, G, sca, scb):
                """Per-block compute; G is the pair super-tile, sca/scb the
                super-chunk offsets of this block's A/B tile ranges."""
                if layer == 1:
                    table_own = hcat1_own
                    WROW, RP, NF, NH = W1ROW, R1, F1, H1
                else:
                    table_own = hcat2_own
                    WROW, RP, NF, NH = W2ROW, R2, NCLS, 1
                sfx = f"L{layer}"
                tba, tbb, tb = TbA[b], TbB[b], Tb[b]
                sc = [sca + t for t in range(tba)] + [
                    scb + t for t in range(tbb)
                ]
                G3 = G[:].rearrange("p (t f) -> p t f", t=PT)

                dl = sb.tile([PB, Tmax], f32, tag="dl")
                nc.sync.dma_start(out=dl[:], in_=dloc_in[b, :, :])
                dlF = sb.tile([1, Tmax * PB], bf16, tag="dlF")
                nc.sync.dma_start(out=dlF[:], in_=dlocF_in[b:b + 1, :])
                own = sb.tile([PB, RP], bf16, tag="own" + sfx)
                nc.sync.dma_start(
                    out=own[:], in_=table_own[b * PB:(b + 1) * PB, 0:RP]
                )

                # one-hot Bm[e, (t, d)] for aggregation
                Bm = sb.tile([PB, Tmax * PB], bf16, tag="Bm")
                Bm3 = Bm[:].rearrange("p (t d) -> p t d", t=Tmax)
                nc.vector.tensor_tensor(
                    out=Bm3[:, 0:tb, :],
                    in0=dl[:][:, 0:tb, None].broadcast_to([PB, tb, PB]),
                    in1=iota_sb[:][:, None, :].broadcast_to([PB, tb, PB]),
                    op=Alu.is_equal,
                )

                # transposed one-hot BmT[d, (t, e)]
                BmT = sb.tile([PB, Tmax * PB], bf16, tag="BmT")
                BmT3 = BmT[:].rearrange("p (t d) -> p t d", t=Tmax)
                for c0 in range(0, tb, MCH):
                    ch = min(MCH, tb - c0)
                    Mc = psC.tile([PB, MCH * PB], f32, tag="Mc")
                    nc.tensor.matmul(
                        out=Mc[:][:, 0:ch * PB],
                        lhsT=ones_sb[:],
                        rhs=dlF[:][:, c0 * PB:(c0 + ch) * PB],
                        start=True,
                        stop=True,
                    )
                    Mc3 = Mc[:].rearrange("p (t d) -> p t d", t=MCH)
                    nc.vector.tensor_tensor(
                        out=BmT3[:, c0:c0 + ch, :],
                        in0=Mc3[:, 0:ch, :],
                        in1=iotaP_sb[:][:, None, :].broadcast_to([PB, ch, PB]),
                        op=Alu.is_equal,
                    )

                # alpha_dst per edge slot: D2[e, (t, h)] = BmT_t.T @ dblk
                D2 = psB.tile([PB, (Tmax + 1) * NH], f32, tag="D2")
                for t in range(tb):
                    nc.tensor.matmul(
                        out=D2[:][:, t * NH:(t + 1) * NH],
                        lhsT=BmT[:][:, t * PB:(t + 1) * PB],
                        rhs=own[:][:, NF + NH:NF + 2 * NH],
                        start=True,
                        stop=True,
                    )

                # logits -> p = exp(leaky_relu(s_src + d_dst))
                sf = sb.tile([PB, Tmax * NH], f32, tag="sf")
                sf3 = sf[:].rearrange("p (t h) -> p t h", t=Tmax)
                nc.vector.tensor_copy(
                    out=sf3[:, 0:tba, :], in_=G3[:, sca:sca + tba, NF:NF + NH]
                )
                nc.vector.tensor_copy(
                    out=sf3[:, tba:tb, :], in_=G3[:, scb:scb + tbb, NF:NF + NH]
                )
                lg = sb.tile([PB, Tmax * NH], f32, tag="lg")
                nc.vector.tensor_tensor(
                    out=lg[:][:, 0:tb * NH],
                    in0=sf[:][:, 0:tb * NH],
                    in1=D2[:][:, 0:tb * NH],
                    op=Alu.add,
                )
                lg2 = sb.tile([PB, Tmax * NH], f32, tag="lg2")
                nc.vector.tensor_scalar_mul(
                    out=lg2[:][:, 0:tb * NH], in0=lg[:][:, 0:tb * NH],
                    scalar1=NEG_SLOPE,
                )
                nc.vector.tensor_tensor(
                    out=lg[:][:, 0:tb * NH],
                    in0=lg[:][:, 0:tb * NH],
                    in1=lg2[:][:, 0:tb * NH],
                    op=Alu.max,
                )
                p = sb.tile([PB, Tmax * NH], bf16, tag="p")
                nc.scalar.activation(
                    out=p[:][:, 0:tb * NH], in_=lg[:][:, 0:tb * NH],
                    func=Act.Exp,
                )
                p3 = p[:].rearrange("p (t h) -> p t h", t=Tmax)

                # in-place: G[:, :, 0:NF] *= p  (A range then B range)
                for r0, rn, t0 in ((sca, tba, 0), (scb, tbb, tba)):
                    if rn == 0:
                        continue
                    out4 = G3[:, r0:r0 + rn, 0:NF].rearrange(
                        "p t (h c) -> p t h c", h=NH
                    )
                    nc.vector.tensor_tensor(
                        out=out4,
                        in0=out4,
                        in1=p3[:, t0:t0 + rn, :, None].broadcast_to(
                            [PB, rn, NH, NF // NH]
                        ),
                        op=Alu.mult,
                    )

                # accumulate out[d] = B.T @ (p*h) and den[d] = B.T @ p
                po = psA.tile([PB, R1], f32, tag="mm")
                for t in range(tb):
                    nc.tensor.matmul(
                        out=po[:][:, 0:NF],
                        lhsT=Bm[:][:, t * PB:(t + 1) * PB],
                        rhs=G3[:, sc[t], 0:NF],
                        start=(t == 0),
                        stop=(t == tb - 1),
                    )
                for t in range(tb):
                    nc.tensor.matmul(
                        out=D2[:][:, Tmax * NH:(Tmax + 1) * NH],
                        lhsT=Bm[:][:, t * PB:(t + 1) * PB],
                        rhs=p[:][:, t * NH:(t + 1) * NH],
                        start=(t == 0),
                        stop=(t == tb - 1),
                    )

                # self loop: p_self = exp(leaky_relu(s_own + d_own))
                sd = sb.tile([PB, NH], f32, tag="sd")
                nc.vector.tensor_tensor(
                    out=sd[:], in0=own[:][:, NF:NF + NH],
                    in1=own[:][:, NF + NH:NF + 2 * NH], op=Alu.add,
                )
                sd2 = sb.tile([PB, NH], f32, tag="sd2")
                nc.vector.tensor_scalar_mul(
                    out=sd2[:], in0=sd[:], scalar1=NEG_SLOPE
                )
                nc.vector.tensor_tensor(
                    out=sd[:], in0=sd[:], in1=sd2[:], op=Alu.max
                )
                pself = sb.tile([PB, NH], f32, tag="pself")
                nc.scalar.activation(out=pself[:], in_=sd[:], func=Act.Exp)
                pselfb = sb.tile([PB, NH], bf16, tag="pselfb")
                nc.vector.tensor_copy(out=pselfb[:], in_=pself[:])

                of = sb.tile([PB, NF], f32, tag="of" + sfx)
                nc.scalar.copy(out=of[:], in_=po[:][:, 0:NF])
                slh = sb.tile([PB, NF], f32, tag="slh" + sfx)
                slh3 = slh[:].rearrange("p (h c) -> p h c", h=NH)
                nc.vector.tensor_tensor(
                    out=slh3,
                    in0=own[:][:, 0:NF].rearrange("p (h c) -> p h c", h=NH),
                    in1=pselfb[:][:, :, None].broadcast_to(
                        [PB, NH, NF // NH]
                    ),
                    op=Alu.mult,
                )
                nc.vector.tensor_tensor(
                    out=of[:], in0=of[:], in1=slh[:], op=Alu.add,
                )
                denf = sb.tile([PB, NH], f32, tag="denf")
                nc.vector.tensor_tensor(
                    out=denf[:],
                    in0=D2[:][:, Tmax * NH:(Tmax + 1) * NH],
                    in1=pself[:], op=Alu.add,
                )
                rden = sb.tile([PB, NH], f32, tag="rden")
                nc.vector.reciprocal(out=rden[:], in_=denf[:])

                o1 = sb.tile([PB, NF], f32, tag="o1" + sfx)
                o13 = o1[:].rearrange("p (h c) -> p h c", h=NH)
                nc.vector.tensor_tensor(
                    out=o13,
                    in0=of[:].rearrange("p (h c) -> p h c", h=NH),
                    in1=rden[:][:, :, None].broadcast_to([PB, NH, NF // NH]),
                    op=Alu.mult,
                )

                if layer == 1:
                    # elu(x) = max(x,0) - 1 + exp(min(x,0)) -> h2 bf16
                    mneg = sb.tile([PB, NF], f32, tag="mneg")
                    nc.vector.tensor_scalar_min(
                        out=mneg[:], in0=o1[:], scalar1=0.0
                    )
                    eneg = sb.tile([PB, NF], f32, tag="eneg")
                    nc.scalar.activation(out=eneg[:], in_=mneg[:], func=Act.Exp)
                    h2a = sb.tile([PB, NF], f32, tag="h2a")
                    nc.vector.tensor_scalar(
                        out=h2a[:], in0=o1[:], scalar1=0.0, scalar2=-1.0,
                        op0=Alu.max, op1=Alu.add,
                    )
                    h2 = sb.tile([PB, NF], bf16, tag="h2")
                    nc.vector.tensor_tensor(
                        out=h2[:], in0=h2a[:], in1=eneg[:], op=Alu.add
                    )
                    for k in range(2):
                        pt = psB.tile([PB, PB], bf16, tag="tr")
                        nc.tensor.transpose(
                            out=pt[:],
                            in_=h2[:][:, k * PB:(k + 1) * PB],
                            identity=ident_sb[:],
                        )
                        nc.scalar.copy(
                            out=h2T_sb[k][:][:, b * PB:(b + 1) * PB],
                            in_=pt[:],
                        )
                    phase_c_block(b)
                else:
                    nc.sync.dma_start(
                        out=out_dev[b * PB:(b + 1) * PB, :],
                        in_=o1[:][:, 0:NCLS],
                    )

            def edge_layer(layer):
                if layer == 1:
                    table_all, WROW = hcat1_all, W1ROW
                else:
                    table_all, WROW = hcat2_all, W2ROW
                sfx = f"L{layer}"
                for pi, pr in enumerate(PAIRS):
                    nA2 = sum(NAb[b] for b in pr)
                    nB2 = sum(NBb[b] for b in pr)
                    tA2 = nA2 // PB
                    tT2 = tA2 + nB2 // PB
                    iA = sb.tile([PB, WA2], i16, tag="iA")
                    nc.sync.dma_start(out=iA[:], in_=idxA_in[pi, :, :])
                    iB = sb.tile([PB, WB2], i16, tag="iB")
                    nc.sync.dma_start(out=iB[:], in_=idxB_in[pi, :, :])

                    G = sb.tile([PB, PT * WROW], bf16, tag="G" + sfx)
                    G3 = G[:].rearrange("p (t f) -> p t f", t=PT)
                    nc.gpsimd.dma_gather(
                        out_ap=G3[:, 0:tA2, :],
                        in_ap=table_all[0:SPLIT, :],
                        idxs_ap=iA[:][:, 0:nA2 // 16],
                        num_idxs=nA2,
                        num_idxs_reg=nA2,
                        elem_size=WROW,
                        single_packet=single_packet,
                    )
                    nc.gpsimd.dma_gather(
                        out_ap=G3[:, tA2:tT2, :],
                        in_ap=table_all[SPLIT:V, :],
                        idxs_ap=iB[:][:, 0:nB2 // 16],
                        num_idxs=nB2,
                        num_idxs_reg=nB2,
                        elem_size=WROW,
                        single_packet=single_packet,
                    )
                    sca, scb = 0, tA2
                    for b in pr:
                        edge_block(layer, b, G, sca, scb)
                        sca += TbA[b]
                        scb += TbB[b]

            with nc.named_scope("gat"):
                # ---------------- phase A: hcat1 = x @ W1cat ----------------
                for nb_i in range(NB):
                    ps = psA.tile([PB, R1], f32, tag="mm")
                    for k in range(2):
                        nc.tensor.matmul(
                            out=ps[:],
                            lhsT=xT_sb[k][:][:, nb_i * PB:(nb_i + 1) * PB],
                            rhs=w1_sb[k][:],
                            start=(k == 0),
                            stop=(k == 1),
                        )
                    hc = sb.tile([PB, R1], bf16, tag="hc1")
                    nc.scalar.copy(out=hc[:], in_=ps[:])
                    nc.sync.dma_start(
                        out=hcat1_own[nb_i * PB:(nb_i + 1) * PB, 0:R1],
                        in_=hc[:],
                    )

                nc.gpsimd.collective_compute(
                    "AllGather",
                    mybir.AluOpType.bypass,
                    replica_groups=groups,
                    ins=[hcat1_own[:, :].opt()],
                    outs=[hcat1_all[:, :].opt()],
                )

                edge_layer(1)   # phase C is interleaved per block

                nc.gpsimd.collective_compute(
                    "AllGather",
                    mybir.AluOpType.bypass,
                    replica_groups=groups,
                    ins=[hcat2_own[:, :].opt()],
                    outs=[hcat2_all[:, :].opt()],
                )

                edge_layer(2)

    nc.compile()
    return nc


# ============================ top-level entry ===============================

def _prepare(inputs):
    x = np.ascontiguousarray(np.asarray(inputs["x"], dtype=np.float32))
    edge_index = np.asarray(inputs["edge_index"], dtype=np.int64)
    w1 = np.asarray(inputs["w1"], dtype=np.float32)
    a_src1 = np.asarray(inputs["a_src1"], dtype=np.float32)
    a_dst1 = np.asarray(inputs["a_dst1"], dtype=np.float32)
    b1 = np.asarray(inputs["b1"], dtype=np.float32)
    w2 = np.asarray(inputs["w2"], dtype=np.float32)
    a_src2 = np.asarray(inputs["a_src2"], dtype=np.float32)
    a_dst2 = np.asarray(inputs["a_dst2"], dtype=np.float32)
    b2 = np.asarray(inputs["b2"], dtype=np.float32)

    assert x.shape == (N_NODES, F1) and edge_index.shape == (2, N_EDGES)
    assert np.all(np.abs(b1) == 0.0), "kernel hardcodes b1 == 0"

    src = edge_index[0]
    dst = edge_index[1]
    perm_row, idxA, idxB, dloc, dlocF, geom = _pack_graph(src, dst)

    w1cat = np.concatenate(
        [w1, w1 @ _expand_heads(a_src1), w1 @ _expand_heads(a_dst1)], axis=1
    ).astype(BF16)
    w2cat = np.concatenate(
        [w2, w2 @ _expand_heads(a_src2), w2 @ _expand_heads(a_dst2)], axis=1
    ).astype(BF16)

    xp = np.zeros((V, F1), dtype=np.float32)
    xp[perm_row] = x
    iota_row = np.broadcast_to(np.arange(PB, dtype=np.float32), (PB, PB)).copy()
    iota_col = np.ascontiguousarray(iota_row.T)
    ident = np.eye(PB, dtype=np.float32).astype(BF16)

    in_maps = []
    xpb = xp.astype(BF16)
    for c in range(N_CORES):
        xT_c = np.ascontiguousarray(xpb[c * SLOTS:(c + 1) * SLOTS].T)
        in_maps.append(
            {
                "xT": xT_c,
                "w1cat": w1cat,
                "w2cat": w2cat,
                "iota_row": iota_row,
                "iota_col": iota_col,
                "ident": ident,
                "idxA": idxA[c],
                "idxB": idxB[c],
                "dloc": dloc[c],
                "dlocF": dlocF[c],
            }
        )
    return in_maps, perm_row, b2, geom


def _assemble(core_outs, perm_row, b2):
    out_all = np.concatenate(core_outs, axis=0)
    out = out_all[perm_row] + b2[None, :]
    return out.astype(np.float32)


def kernel(**inputs) -> np.ndarray:
    in_maps, perm_row, b2, geom = _prepare(inputs)

    import concourse.bass_utils as bass_utils

    key = ("nc", tuple(sorted(geom.items())))
    if key not in _CACHE:
        _CACHE.clear()
        _CACHE[key] = _build_program(geom)
    nc = _CACHE[key]

    trace = bool(int(os.environ.get("GAT_TRACE", "0")))
    res = bass_utils.run_bass_kernel_spmd(
        nc,
        in_maps,
        core_ids=list(range(N_CORES)),
        trace=trace,
        trace_cores=list(range(N_CORES)) if trace else None,
        stitch_traces=trace,
    )
    _CACHE["last_results"] = res

    return _assemble([r["out_dev"] for r in res.results], perm_row, b2)


# revision 17
# speedup vs baseline: 3.9133x; 1.2154x over previous
"""Two-layer GAT (PyG semantics) on 8 Trainium2 NeuronCores.

Gather-lean bf16 design.  The original baseline spent 4.2 of 5.1 ms on
GPSIMD Q7 descriptor generation for dma_gather (~8.5 ns per gathered
row, 502k rows per core).  This version:

  * No self-loop edges in the edge list: the self-loop term p_self*h[own]
    is added per dst block from the locally-stored rows.
  * No per-edge dst-alpha gather (50% of the baseline's indices): the
    dst alpha is broadcast per block on the PE instead - a one-partition
    matmul replicates the per-edge dst-slot ids down 128 partitions, DVE
    is_equal against a column-iota builds the transposed one-hot
    BmT[d, e], and BmT_t.T @ dblk gives alpha_dst per edge slot.
  * The softmax denominator comes from an extra tiny PE accumulation
    Bm_t.T @ p (contiguous rhs) instead of appending p to the gathered
    rows (which needed a slow strided DVE scatter).
  * bf16 node tables: layer-1 rows [h(256)|s(8)|d(8)] at a 768 B stride,
    layer-2 rows [h2(40)|s|d] at 256 B.  Gathers move real edges only
    (per-block counts baked at trace time as the max across cores, ceil
    16; gather tails are memset and killed by zero one-hot columns).
  * bf16 PE matmuls (1 cycle/row vs 4 for fp32) with f32 PSUM accum.
  * leaky_relu runs on the scalar engine as Prelu(alpha) - it shares the
    activation table set with Exp, so no table-switch cost (Lrelu
    IGNORES alpha and hardcodes slope 0.01; Prelu honors it).
  * Layer-2 projection (phase C) is interleaved into the layer-1 edge
    loop per block so only the AllGather separates the two layers.

Host: assign nodes to 8 cores balancing in-edges, pack each core's
nodes into 49 blocks of 128 slots; route edges to the block owning
their dst, split by src table half (int16 gather indices address
<=32768 rows; the table splits at row 25088), sorted by src row.
Device: hcat = x @ [W|W@a_src|W@a_dst] per core, AllGather to the full
table, per-block gather + softmax + one-hot-matmul aggregation + ELU,
then the same again for layer 2.  Host inverse-permutes and adds b2.
"""

import os

import numpy as np

try:
    import ml_dtypes

    BF16 = ml_dtypes.bfloat16
except ImportError:  # pragma: no cover
    BF16 = np.float32

# ---------------- geometry (hardcoded for nn_GAT_51694226374713) ------------
N_NODES = 50000
N_EDGES = 800000
N_CORES = 8
NB = 49                    # dst blocks per core
PB = 128                   # dst nodes (slots) per block
SLOTS = NB * PB            # 6272 node slots per core
V = N_CORES * SLOTS        # 50176 rows in the gathered node tables
SPLIT = (N_CORES // 2) * SLOTS   # table half boundary (row 25088)
CAP_HALF = 1280            # pass-2 per-(block, half) edge cap
F1 = 256                   # input features
H1, C1 = 8, 32             # layer-1 heads x channels
R1 = F1 + 2 * H1           # 272: layer-1 row payload  h | s | d
W1ROW = 384                # layer-1 row stride in bf16 elems (768 B)
NCLS = 40
R2 = NCLS + 2              # 42: layer-2 row payload
W2ROW = 128                # layer-2 row stride in bf16 elems (256 B)
NEG_SLOPE = 0.2
TAIL = 999.0               # dst-slot sentinel for pad edge slots

_CACHE: dict = {}


# ============================ host preprocessing ============================

def _greedy_pack(items, weights_list, caps_list, slot_caps):
    """Place items (ordered) into bins; weights_list/caps_list are parallel
    lists of per-item weight arrays and per-bin capacity arrays.  Returns
    bin_of_item.  Greedy: emptiest bin (by total weight) first, skipping
    bins where any cap or the slot cap would overflow."""
    import heapq

    n_bins = len(slot_caps)
    used = [np.zeros(n_bins, dtype=np.int64) for _ in weights_list]
    slots_used = np.zeros(n_bins, dtype=np.int64)
    total = np.zeros(n_bins, dtype=np.int64)
    bin_of = {}
    heap = [(0, b) for b in range(n_bins)]
    heapq.heapify(heap)
    for it in items:
        ws = [w[it] for w in weights_list]
        stash = []
        while True:
            if not heap:
                raise RuntimeError("packing failed; raise CAP_HALF")
            t, b = heapq.heappop(heap)
            if t != total[b]:
                continue  # stale
            if slots_used[b] >= slot_caps[b]:
                continue  # permanently full
            if any(
                used[k][b] + ws[k] > caps_list[k][b] for k in range(len(ws))
            ):
                stash.append((t, b))
                continue
            bin_of[it] = b
            slots_used[b] += 1
            for k in range(len(ws)):
                used[k][b] += ws[k]
            total[b] += sum(ws)
            heapq.heappush(heap, (int(total[b]), b))
            break
        for item in stash:
            heapq.heappush(heap, item)
    return bin_of


def _wrap_idx(lin):
    """Linear index array [n] (n % 16 == 0) -> dma_gather layout
    [128, n // 16] int16 (16-partition wrap, replicated to 128)."""
    n = lin.size
    assert n % 16 == 0
    w = lin.reshape(n // 16, 16).T.astype(np.int16)  # [16, n/16]
    return np.ascontiguousarray(np.tile(w, (8, 1)))  # [128, n/16]


def _ceil16(x):
    return max((int(x) + 15) // 16 * 16, 16)


def _pack_graph(src, dst):
    """Assign nodes to (core, block, slot); route real edges (no self
    loops).  Returns perm_row [N] plus per-core device arrays and the
    baked per-block geometry (max over cores)."""
    deg = np.bincount(dst, minlength=N_NODES)

    # ---- pass 1: nodes -> cores, balancing total in-edges ----
    order = np.argsort(-deg, kind="stable")
    core_of = _greedy_pack(
        order,
        [deg],
        [np.full(N_CORES, 1 << 60, dtype=np.int64)],
        np.full(N_CORES, SLOTS, dtype=np.int64),
    )
    node_core = np.empty(N_NODES, dtype=np.int64)
    for nd, c in core_of.items():
        node_core[nd] = c

    half_b_src = node_core[src] >= (N_CORES // 2)
    degA = np.bincount(dst[~half_b_src], minlength=N_NODES)
    degB = np.bincount(dst[half_b_src], minlength=N_NODES)

    # ---- pass 2: per core, nodes -> blocks with per-half edge caps ----
    node_bin = np.empty(N_NODES, dtype=np.int64)
    node_slot = np.zeros(N_NODES, dtype=np.int64)
    for c in range(N_CORES):
        nodes_c = np.where(node_core == c)[0]
        ordc = nodes_c[np.argsort(-(deg[nodes_c]), kind="stable")]
        bin_of = _greedy_pack(
            ordc,
            [degA, degB],
            [
                np.full(NB, CAP_HALF, dtype=np.int64),
                np.full(NB, CAP_HALF, dtype=np.int64),
            ],
            np.full(NB, PB, dtype=np.int64),
        )
        # relabel bins by descending edge count so block b has similar
        # size on every core (per-block counts are baked as cross-core
        # maxima; aligned quantiles keep the padding small)
        btot = np.zeros(NB, dtype=np.int64)
        for nd in ordc:
            btot[bin_of[nd]] += deg[nd]
        rank = np.empty(NB, dtype=np.int64)
        rank[np.argsort(-btot, kind="stable")] = np.arange(NB)
        slots_used = np.zeros(NB, dtype=np.int64)
        for nd in ordc:
            b = rank[bin_of[nd]]
            node_bin[nd] = c * NB + b
            node_slot[nd] = slots_used[b]
            slots_used[b] += 1

    perm_row = (node_bin * PB + node_slot).astype(np.int64)

    # ---- edge routing: per (bin, half), sorted by src row ----
    n_bins = N_CORES * NB
    ebin = node_bin[dst]
    src_row_e = perm_row[src]
    dst_slot_e = perm_row[dst] % PB
    keyhalf = half_b_src.astype(np.int64)
    sort_idx = np.lexsort((src_row_e, keyhalf, ebin))
    ebin_s = ebin[sort_idx]
    half_s = keyhalf[sort_idx]
    src_s = src_row_e[sort_idx]
    dsl_s = dst_slot_e[sort_idx]

    grp = ebin_s * 2 + half_s
    counts = np.bincount(grp, minlength=n_bins * 2)
    realA = counts[0::2].reshape(N_CORES, NB)
    realB = counts[1::2].reshape(N_CORES, NB)
    assert realA.max() <= CAP_HALF and realB.max() <= CAP_HALF

    # baked per-block geometry: max over cores, ceil 16
    NAb = np.array([_ceil16(realA[:, b].max()) for b in range(NB)])
    NBb = np.array([_ceil16(realB[:, b].max()) for b in range(NB)])
    TbA = (NAb + PB - 1) // PB
    TbB = (NBb + PB - 1) // PB
    Tb = TbA + TbB
    TAmax = int(TbA.max())
    TBmax = int(TbB.max())
    Tmax = int(Tb.max())

    starts = np.zeros(n_bins * 2 + 1, dtype=np.int64)
    np.cumsum(counts, out=starts[1:])
    pos = np.arange(ebin_s.size) - starts[grp]

    b_of_bin = np.arange(n_bins) % NB
    # linear edge slot j within the block: A at [0, NAb), B at TbA*128 +
    j = np.where(half_s == 0, pos, (TbA[b_of_bin] * PB)[ebin_s] + pos)

    linA = np.zeros((n_bins, TAmax * PB), dtype=np.int64)
    linB = np.zeros((n_bins, TBmax * PB), dtype=np.int64)
    dlocF = np.full((n_bins, Tmax * PB), TAIL, dtype=np.float32)

    mA = half_s == 0
    linA[ebin_s[mA], pos[mA]] = src_s[mA]
    linB[ebin_s[~mA], pos[~mA]] = src_s[~mA] - SPLIT
    dlocF[ebin_s, j] = dsl_s

    WA = max(NAb) // 16
    WB = max(NBb) // 16
    idxA = np.zeros((N_CORES, NB, PB, WA), dtype=np.int16)
    idxB = np.zeros((N_CORES, NB, PB, WB), dtype=np.int16)
    for bi in range(n_bins):
        c, b = bi // NB, bi % NB
        wa = _wrap_idx(linA[bi, : NAb[b]])
        idxA[c, b, :, : wa.shape[1]] = wa
        wb = _wrap_idx(linB[bi, : NBb[b]])
        idxB[c, b, :, : wb.shape[1]] = wb

    # dloc [c, b, p, t] = dlocF[c, b, t*128 + p]
    dloc = np.ascontiguousarray(
        dlocF.reshape(N_CORES, NB, Tmax, PB).transpose(0, 1, 3, 2)
    ).astype(BF16)
    dlocF = dlocF.reshape(N_CORES, NB, Tmax * PB).astype(BF16)

    geom = {
        "NAb": tuple(int(x) for x in NAb),
        "NBb": tuple(int(x) for x in NBb),
        "TbA": tuple(int(x) for x in TbA),
        "TbB": tuple(int(x) for x in TbB),
        "Tb": tuple(int(x) for x in Tb),
        "TAmax": TAmax,
        "TBmax": TBmax,
        "Tmax": Tmax,
        "WA": WA,
        "WB": WB,
    }
    return perm_row, idxA, idxB, dloc, dlocF, geom


def _expand_heads(a):
    """[H, C] attention vector -> block-diagonal [H*C, H] matrix."""
    h, c = a.shape
    m = np.zeros((h * c, h), dtype=np.float32)
    for i in range(h):
        m[i * c:(i + 1) * c, i] = a[i]
    return m


# ============================ device program ================================

def _build_program(geom):
    import concourse.bacc as bacc
    import concourse.mybir as mybir
    import concourse.tile as tile

    f32 = mybir.dt.float32
    bf16 = mybir.dt.bfloat16
    i16 = mybir.dt.int16
    Alu = mybir.AluOpType
    Act = mybir.ActivationFunctionType

    NAb, NBb = geom["NAb"], geom["NBb"]
    TbA, TbB, Tb = geom["TbA"], geom["TbB"], geom["Tb"]
    Tmax = geom["Tmax"]
    WA, WB = geom["WA"], geom["WB"]
    single_packet = bool(int(os.environ.get("GAT_SP", "0")))
    MCH = 4                    # BmT build chunk; 4*128 = 512 = max mm free dim
    DEPTH = int(os.environ.get("GAT_DEPTH", "3"))

    nc = bacc.Bacc(
        "TRN2", target_bir_lowering=False, debug=False, num_devices=N_CORES
    )

    # ---- kernel I/O ----
    xT = nc.dram_tensor("xT", [F1, SLOTS], bf16, kind="ExternalInput")
    w1cat = nc.dram_tensor("w1cat", [F1, R1], bf16, kind="ExternalInput")
    w2cat = nc.dram_tensor("w2cat", [F1, R2], bf16, kind="ExternalInput")
    iota_in = nc.dram_tensor("iota_row", [PB, PB], bf16, kind="ExternalInput")
    iotaP_in = nc.dram_tensor("iota_col", [PB, PB], f32, kind="ExternalInput")
    ident_in = nc.dram_tensor("ident", [PB, PB], bf16, kind="ExternalInput")
    idxA_in = nc.dram_tensor("idxA", [NB, PB, WA], i16, kind="ExternalInput")
    idxB_in = nc.dram_tensor("idxB", [NB, PB, WB], i16, kind="ExternalInput")
    dloc_in = nc.dram_tensor("dloc", [NB, PB, Tmax], bf16, kind="ExternalInput")
    dlocF_in = nc.dram_tensor(
        "dlocF", [NB, Tmax * PB], bf16, kind="ExternalInput"
    )
    out_dev = nc.dram_tensor("out_dev", [SLOTS, NCLS], f32, kind="ExternalOutput")

    # ---- internal DRAM ----
    hcat1_own = nc.dram_tensor("hcat1_own", [SLOTS, W1ROW], bf16, kind="Internal")
    hcat1_all = nc.dram_tensor(
        "hcat1_all", [V, W1ROW], bf16, kind="Internal", addr_space="Shared"
    )
    hcat2_own = nc.dram_tensor("hcat2_own", [SLOTS, W2ROW], bf16, kind="Internal")
    hcat2_all = nc.dram_tensor(
        "hcat2_all", [V, W2ROW], bf16, kind="Internal", addr_space="Shared"
    )

    groups = [list(range(N_CORES))]

    with tile.TileContext(nc) as tc:
        with (
            tc.tile_pool(name="persist", bufs=1) as pp,
            tc.tile_pool(name="sb", bufs=2) as sb,
            tc.tile_pool(name="psA", bufs=2, space="PSUM") as psA,
            tc.tile_pool(name="psB", bufs=2, space="PSUM") as psB,
        ):
            # ---------------- persistent tiles ----------------
            iota_sb = pp.tile([PB, PB], bf16, tag="iota")
            nc.sync.dma_start(out=iota_sb[:], in_=iota_in[:, :])
            iotaP_sb = pp.tile([PB, PB], f32, tag="iotaP")
            nc.sync.dma_start(out=iotaP_sb[:], in_=iotaP_in[:, :])
            ident_sb = pp.tile([PB, PB], bf16, tag="ident")
            nc.sync.dma_start(out=ident_sb[:], in_=ident_in[:, :])
            ones_sb = pp.tile([1, PB], bf16, tag="ones")
            nc.vector.memset(ones_sb[:], 1.0)

            w1_sb = [
                pp.tile([PB, R1], bf16, tag=f"w1_{k}", name=f"w1_sb{k}")
                for k in range(2)
            ]
            for k in range(2):
                nc.sync.dma_start(out=w1_sb[k][:], in_=w1cat[k * PB:(k + 1) * PB, :])
            w2_sb = [
                pp.tile([PB, R2], bf16, tag=f"w2_{k}", name=f"w2_sb{k}")
                for k in range(2)
            ]
            for k in range(2):
                nc.sync.dma_start(out=w2_sb[k][:], in_=w2cat[k * PB:(k + 1) * PB, :])

            xT_sb = [
                pp.tile([PB, SLOTS], bf16, tag=f"xT{k}", name=f"xT_sb{k}")
                for k in range(2)
            ]
            for k in range(2):
                nc.sync.dma_start(out=xT_sb[k][:], in_=xT[k * PB:(k + 1) * PB, :])
            h2T_sb = [
                pp.tile([PB, SLOTS], bf16, tag=f"h2T{k}", name=f"h2T_sb{k}")
                for k in range(2)
            ]

            def phase_c_block(nb_i):
                ps = psA.tile([PB, R1], f32, tag="mm")
                for k in range(2):
                    nc.tensor.matmul(
                        out=ps[:][:, 0:R2],
                        lhsT=h2T_sb[k][:][:, nb_i * PB:(nb_i + 1) * PB],
                        rhs=w2_sb[k][:],
                        start=(k == 0),
                        stop=(k == 1),
                    )
                hc2 = sb.tile([PB, R2], bf16, tag="hc2")
                nc.scalar.copy(out=hc2[:], in_=ps[:][:, 0:R2])
                nc.sync.dma_start(
                    out=hcat2_own[nb_i * PB:(nb_i + 1) * PB, 0:R2],
                    in_=hc2[:],
                )

            def edge_layer(layer):
                if layer == 1:
                    table_all, table_own = hcat1_all, hcat1_own
                    WROW, RP, NF, NH = W1ROW, R1, F1, H1
                else:
                    table_all, table_own = hcat2_all, hcat2_own
                    WROW, RP, NF, NH = W2ROW, R2, NCLS, 1
                sfx = f"L{layer}"
                for b in range(NB):
                    tba, tbb, tb = TbA[b], TbB[b], Tb[b]
                    na, nb_ = NAb[b], NBb[b]
                    iA = sb.tile([PB, WA], i16, tag="iA", bufs=DEPTH)
                    nc.sync.dma_start(out=iA[:], in_=idxA_in[b, :, :])
                    iB = sb.tile([PB, WB], i16, tag="iB", bufs=DEPTH)
                    nc.sync.dma_start(out=iB[:], in_=idxB_in[b, :, :])
                    dl = sb.tile([PB, Tmax], bf16, tag="dl", bufs=DEPTH)
                    nc.sync.dma_start(out=dl[:], in_=dloc_in[b, :, :])
                    dlF = sb.tile([1, Tmax * PB], bf16, tag="dlF", bufs=DEPTH)
                    nc.sync.dma_start(out=dlF[:], in_=dlocF_in[b:b + 1, :])
                    own = sb.tile([PB, RP], bf16, tag="own" + sfx, bufs=DEPTH)
                    nc.sync.dma_start(
                        out=own[:], in_=table_own[b * PB:(b + 1) * PB, 0:RP]
                    )

                    G = sb.tile(
                        [PB, Tmax * WROW], bf16, tag="G" + sfx, bufs=DEPTH
                    )
                    G3 = G[:].rearrange("p (t f) -> p t f", t=Tmax)
                    # zero the partial tail tiles (junk killed by zero
                    # one-hot columns, but must stay finite)
                    nc.vector.memset(G3[:, tba - 1, :], 0.0)
                    nc.vector.memset(G3[:, tb - 1, :], 0.0)
                    nc.gpsimd.dma_gather(
                        out_ap=G3[:, 0:tba, :],
                        in_ap=table_all[0:SPLIT, :],
                        idxs_ap=iA[:][:, 0:na // 16],
                        num_idxs=na,
                        num_idxs_reg=na,
                        elem_size=WROW,
                        single_packet=single_packet,
                    )
                    nc.gpsimd.dma_gather(
                        out_ap=G3[:, tba:tb, :],
                        in_ap=table_all[SPLIT:V, :],
                        idxs_ap=iB[:][:, 0:nb_ // 16],
                        num_idxs=nb_,
                        num_idxs_reg=nb_,
                        elem_size=WROW,
                        single_packet=single_packet,
                    )

                    # one-hot Bm[e, (t, d)] for aggregation
                    Bm = sb.tile([PB, Tmax * PB], bf16, tag="Bm", bufs=DEPTH)
                    Bm3 = Bm[:].rearrange("p (t d) -> p t d", t=Tmax)
                    nc.vector.tensor_tensor(
                        out=Bm3[:, 0:tb, :],
                        in0=dl[:][:, 0:tb, None].broadcast_to([PB, tb, PB]),
                        in1=iota_sb[:][:, None, :].broadcast_to([PB, tb, PB]),
                        op=Alu.is_equal,
                    )

                    # transposed one-hot BmT[d, (t, e)]
                    BmT = sb.tile([PB, Tmax * PB], bf16, tag="BmT", bufs=DEPTH)
                    BmT3 = BmT[:].rearrange("p (t d) -> p t d", t=Tmax)
                    for c0 in range(0, tb, MCH):
                        ch = min(MCH, tb - c0)
                        Mc = psB.tile([PB, MCH * PB], f32, tag="Mc")
                        nc.tensor.matmul(
                            out=Mc[:][:, 0:ch * PB],
                            lhsT=ones_sb[:],
                            rhs=dlF[:][:, c0 * PB:(c0 + ch) * PB],
                            start=True,
                            stop=True,
                        )
                        Mc3 = Mc[:].rearrange("p (t d) -> p t d", t=MCH)
                        nc.vector.tensor_tensor(
                            out=BmT3[:, c0:c0 + ch, :],
                            in0=Mc3[:, 0:ch, :],
                            in1=iotaP_sb[:][:, None, :].broadcast_to(
                                [PB, ch, PB]
                            ),
                            op=Alu.is_equal,
                        )

                    # alpha_dst per edge slot: D2[e, (t, h)] = BmT_t.T @ dblk
                    D2 = psB.tile([PB, (Tmax + 1) * NH], f32, tag="D2")
                    for t in range(tb):
                        nc.tensor.matmul(
                            out=D2[:][:, t * NH:(t + 1) * NH],
                            lhsT=BmT[:][:, t * PB:(t + 1) * PB],
                            rhs=own[:][:, NF + NH:NF + 2 * NH],
                            start=True,
                            stop=True,
                        )

                    # logits -> p = exp(leaky_relu(s_src + d_dst))
                    sf = sb.tile([PB, Tmax * NH], f32, tag="sf", bufs=DEPTH)
                    sf3 = sf[:].rearrange("p (t h) -> p t h", t=Tmax)
                    nc.scalar.copy(
                        out=sf3[:, 0:tb, :], in_=G3[:, 0:tb, NF:NF + NH]
                    )
                    lg = sb.tile([PB, Tmax * NH], f32, tag="lg", bufs=DEPTH)
                    nc.vector.tensor_tensor(
                        out=lg[:][:, 0:tb * NH],
                        in0=sf[:][:, 0:tb * NH],
                        in1=D2[:][:, 0:tb * NH],
                        op=Alu.add,
                    )
                    nc.scalar.activation(
                        out=lg[:][:, 0:tb * NH], in_=lg[:][:, 0:tb * NH],
                        func=Act.Prelu, alpha=NEG_SLOPE,
                    )
                    p = sb.tile([PB, Tmax * NH], bf16, tag="p", bufs=DEPTH)
                    nc.scalar.activation(
                        out=p[:][:, 0:tb * NH], in_=lg[:][:, 0:tb * NH],
                        func=Act.Exp,
                    )
                    p3 = p[:].rearrange("p (t h) -> p t h", t=Tmax)

                    # in-place: G[:, :, 0:NF] *= p
                    out4 = G3[:, 0:tb, 0:NF].rearrange(
                        "p t (h c) -> p t h c", h=NH
                    )
                    nc.vector.tensor_tensor(
                        out=out4,
                        in0=out4,
                        in1=p3[:, 0:tb, :, None].broadcast_to(
                            [PB, tb, NH, NF // NH]
                        ),
                        op=Alu.mult,
                    )

                    # accumulate out[d] = B.T @ (p*h) and den[d] = B.T @ p
                    po = psA.tile([PB, R1], f32, tag="mm")
                    for t in range(tb):
                        nc.tensor.matmul(
                            out=po[:][:, 0:NF],
                            lhsT=Bm[:][:, t * PB:(t + 1) * PB],
                            rhs=G3[:, t, 0:NF],
                            start=(t == 0),
                            stop=(t == tb - 1),
                        )
                    for t in range(tb):
                        nc.tensor.matmul(
                            out=D2[:][:, Tmax * NH:(Tmax + 1) * NH],
                            lhsT=Bm[:][:, t * PB:(t + 1) * PB],
                            rhs=p[:][:, t * NH:(t + 1) * NH],
                            start=(t == 0),
                            stop=(t == tb - 1),
                        )

                    # self loop: p_self = exp(leaky_relu(s_own + d_own))
                    sd = sb.tile([PB, NH], f32, tag="sd")
                    nc.vector.tensor_tensor(
                        out=sd[:], in0=own[:][:, NF:NF + NH],
                        in1=own[:][:, NF + NH:NF + 2 * NH], op=Alu.add,
                    )
                    nc.scalar.activation(
                        out=sd[:], in_=sd[:], func=Act.Prelu, alpha=NEG_SLOPE
                    )
                    pself = sb.tile([PB, NH], f32, tag="pself")
                    nc.scalar.activation(out=pself[:], in_=sd[:], func=Act.Exp)
                    pselfb = sb.tile([PB, NH], bf16, tag="pselfb")
                    nc.scalar.copy(out=pselfb[:], in_=pself[:])

                    of = sb.tile([PB, NF], f32, tag="of" + sfx)
                    nc.scalar.copy(out=of[:], in_=po[:][:, 0:NF])
                    slh = sb.tile([PB, NF], f32, tag="slh" + sfx)
                    slh3 = slh[:].rearrange("p (h c) -> p h c", h=NH)
                    nc.vector.tensor_tensor(
                        out=slh3,
                        in0=own[:][:, 0:NF].rearrange("p (h c) -> p h c", h=NH),
                        in1=pselfb[:][:, :, None].broadcast_to(
                            [PB, NH, NF // NH]
                        ),
                        op=Alu.mult,
                    )
                    nc.vector.tensor_tensor(
                        out=of[:], in0=of[:], in1=slh[:], op=Alu.add,
                    )
                    denf = sb.tile([PB, NH], f32, tag="denf")
                    nc.vector.tensor_tensor(
                        out=denf[:],
                        in0=D2[:][:, Tmax * NH:(Tmax + 1) * NH],
                        in1=pself[:], op=Alu.add,
                    )
                    rden = sb.tile([PB, NH], f32, tag="rden")
                    nc.vector.reciprocal(out=rden[:], in_=denf[:])

                    o1 = sb.tile([PB, NF], f32, tag="o1" + sfx)
                    o13 = o1[:].rearrange("p (h c) -> p h c", h=NH)
                    nc.vector.tensor_tensor(
                        out=o13,
                        in0=of[:].rearrange("p (h c) -> p h c", h=NH),
                        in1=rden[:][:, :, None].broadcast_to(
                            [PB, NH, NF // NH]
                        ),
                        op=Alu.mult,
                    )

                    if layer == 1:
                        # elu(x) = max(x,0) - 1 + exp(min(x,0)) -> h2 bf16
                        mneg = sb.tile([PB, NF], f32, tag="mneg")
                        nc.vector.tensor_scalar_min(
                            out=mneg[:], in0=o1[:], scalar1=0.0
                        )
                        eneg = sb.tile([PB, NF], f32, tag="eneg")
                        nc.scalar.activation(
                            out=eneg[:], in_=mneg[:], func=Act.Exp
                        )
                        h2a = sb.tile([PB, NF], f32, tag="h2a")
                        nc.vector.tensor_scalar(
                            out=h2a[:], in0=o1[:], scalar1=0.0, scalar2=-1.0,
                            op0=Alu.max, op1=Alu.add,
                        )
                        h2 = sb.tile([PB, NF], bf16, tag="h2")
                        nc.vector.tensor_tensor(
                            out=h2[:], in0=h2a[:], in1=eneg[:], op=Alu.add
                        )
                        for k in range(2):
                            pt = psB.tile([PB, PB], bf16, tag="tr")
                            nc.tensor.transpose(
                                out=pt[:],
                                in_=h2[:][:, k * PB:(k + 1) * PB],
                                identity=ident_sb[:],
                            )
                            nc.scalar.copy(
                                out=h2T_sb[k][:][:, b * PB:(b + 1) * PB],
                                in_=pt[:],
                            )
                        phase_c_block(b)
                    else:
                        nc.sync.dma_start(
                            out=out_dev[b * PB:(b + 1) * PB, :],
                            in_=o1[:][:, 0:NCLS],
                        )

            with nc.named_scope("gat"):
                # ---------------- phase A: hcat1 = x @ W1cat ----------------
                for nb_i in range(NB):
                    ps = psA.tile([PB, R1], f32, tag="mm")
                    for k in range(2):
                        nc.tensor.matmul(
                            out=ps[:],
                            lhsT=xT_sb[k][:][:, nb_i * PB:(nb_i + 1) * PB],
                            rhs=w1_sb[k][:],
                            start=(k == 0),
                            stop=(k == 1),
                        )
                    hc = sb.tile([PB, R1], bf16, tag="hc1")
                    nc.scalar.copy(out=hc[:], in_=ps[:])
                    nc.sync.dma_start(
                        out=hcat1_own[nb_i * PB:(nb_i + 1) * PB, 0:R1],
                        in_=hc[:],
                    )

                nc.gpsimd.collective_compute(
                    "AllGather",
                    mybir.AluOpType.bypass,
                    replica_groups=groups,
                    ins=[hcat1_own[:, :].opt()],
                    outs=[hcat1_all[:, :].opt()],
                )

                edge_layer(1)   # phase C interleaved per block

                nc.gpsimd.collective_compute(
                    "AllGather",
                    mybir.AluOpType.bypass,
                    replica_groups=groups,
                    ins=[hcat2_own[:, :].opt()],
                    outs=[hcat2_all[:, :].opt()],
                )

                edge_layer(2)

    nc.compile()
    return nc


# ============================ top-level entry ===============================

def _prepare(inputs):
    x = np.ascontiguousarray(np.asarray(inputs["x"], dtype=np.float32))
    edge_index = np.asarray(inputs["edge_index"], dtype=np.int64)
    w1 = np.asarray(inputs["w1"], dtype=np.float32)
    a_src1 = np.asarray(inputs["a_src1"], dtype=np.float32)
    a_dst1 = np.asarray(inputs["a_dst1"], dtype=np.float32)
    b1 = np.asarray(inputs["b1"], dtype=np.float32)
    w2 = np.asarray(inputs["w2"], dtype=np.float32)
    a_src2 = np.asarray(inputs["a_src2"], dtype=np.float32)
    a_dst2 = np.asarray(inputs["a_dst2"], dtype=np.float32)
    b2 = np.asarray(inputs["b2"], dtype=np.float32)

    assert x.shape == (N_NODES, F1) and edge_index.shape == (2, N_EDGES)
    assert np.all(np.abs(b1) == 0.0), "kernel hardcodes b1 == 0"

    src = edge_index[0]
    dst = edge_index[1]
    perm_row, idxA, idxB, dloc, dlocF, geom = _pack_graph(src, dst)

    w1cat = np.concatenate(
        [w1, w1 @ _expand_heads(a_src1), w1 @ _expand_heads(a_dst1)], axis=1
    ).astype(BF16)
    w2cat = np.concatenate(
        [w2, w2 @ _expand_heads(a_src2), w2 @ _expand_heads(a_dst2)], axis=1
    ).astype(BF16)

    xp = np.zeros((V, F1), dtype=np.float32)
    xp[perm_row] = x
    iota_f = np.broadcast_to(np.arange(PB, dtype=np.float32), (PB, PB))
    iota_row = np.ascontiguousarray(iota_f).astype(BF16)
    iota_col = np.ascontiguousarray(iota_f.T)
    ident = np.eye(PB, dtype=np.float32).astype(BF16)

    xpb = xp.astype(BF16)
    in_maps = []
    for c in range(N_CORES):
        xT_c = np.ascontiguousarray(xpb[c * SLOTS:(c + 1) * SLOTS].T)
        in_maps.append(
            {
                "xT": xT_c,
                "w1cat": w1cat,
                "w2cat": w2cat,
                "iota_row": iota_row,
                "iota_col": iota_col,
                "ident": ident,
                "idxA": idxA[c],
                "idxB": idxB[c],
                "dloc": dloc[c],
                "dlocF": dlocF[c],
            }
        )
    return in_maps, perm_row, b2, geom


def _assemble(core_outs, perm_row, b2):
    out_all = np.concatenate(core_outs, axis=0)
    out = out_all[perm_row] + b2[None, :]
    return out.astype(np.float32)


def kernel(**inputs) -> np.ndarray:
    in_maps, perm_row, b2, geom = _prepare(inputs)

    import concourse.bass_utils as bass_utils

    key = ("nc", tuple(sorted(geom.items())))
    if key not in _CACHE:
        _CACHE.clear()
        _CACHE[key] = _build_program(geom)
    nc = _CACHE[key]

    trace = bool(int(os.environ.get("GAT_TRACE", "0")))
    res = bass_utils.run_bass_kernel_spmd(
        nc,
        in_maps,
        core_ids=list(range(N_CORES)),
        trace=trace,
        trace_cores=list(range(N_CORES)) if trace else None,
        stitch_traces=trace,
    )
    _CACHE["last_results"] = res

    return _assemble([r["out_dev"] for r in res.results], perm_row, b2)


# revision 18
# speedup vs baseline: 3.9766x; 1.0162x over previous
"""Two-layer GAT (PyG semantics) on 8 Trainium2 NeuronCores.

Gather-lean bf16 design.  The original baseline spent 4.2 of 5.1 ms on
GPSIMD Q7 descriptor generation for dma_gather (~8.5 ns per gathered
row, 502k rows per core).  This version:

  * No self-loop edges in the edge list: the self-loop term p_self*h[own]
    is added per dst block from the locally-stored rows.
  * No per-edge dst-alpha gather (50% of the baseline's indices): the
    dst alpha is broadcast per block on the PE instead - a one-partition
    matmul replicates the per-edge dst-slot ids down 128 partitions, DVE
    is_equal against a column-iota builds the transposed one-hot
    BmT[d, e], and BmT_t.T @ dblk gives alpha_dst per edge slot.
  * The softmax denominator comes from an extra tiny PE accumulation
    Bm_t.T @ p (contiguous rhs) instead of appending p to the gathered
    rows (which needed a slow strided DVE scatter).
  * bf16 node tables: layer-1 rows [h(256)|s(8)|d(8)] at a 768 B stride,
    layer-2 rows [h2(40)|s|d] at 256 B.  Gathers move real edges only
    (per-block counts baked at trace time as the max across cores, ceil
    16; gather tails are memset and killed by zero one-hot columns).
  * bf16 PE matmuls (1 cycle/row vs 4 for fp32) with f32 PSUM accum.
  * leaky_relu runs on the scalar engine as Prelu(alpha) - it shares the
    activation table set with Exp, so no table-switch cost (Lrelu
    IGNORES alpha and hardcodes slope 0.01; Prelu honors it).
  * Layer-2 projection (phase C) is interleaved into the layer-1 edge
    loop per block so only the AllGather separates the two layers.

Host: assign nodes to 8 cores balancing in-edges, pack each core's
nodes into 49 blocks of 128 slots; route edges to the block owning
their dst, split by src table half (int16 gather indices address
<=32768 rows; the table splits at row 25088), sorted by src row.
Device: hcat = x @ [W|W@a_src|W@a_dst] per core, AllGather to the full
table, per-block gather + softmax + one-hot-matmul aggregation + ELU,
then the same again for layer 2.  Host inverse-permutes and adds b2.
"""

import os

import numpy as np

try:
    import ml_dtypes

    BF16 = ml_dtypes.bfloat16
except ImportError:  # pragma: no cover
    BF16 = np.float32

# ---------------- geometry (hardcoded for nn_GAT_51694226374713) ------------
N_NODES = 50000
N_EDGES = 800000
N_CORES = 8
NB = 49                    # dst blocks per core
PB = 128                   # dst nodes (slots) per block
SLOTS = NB * PB            # 6272 node slots per core
V = N_CORES * SLOTS        # 50176 rows in the gathered node tables
SPLIT = (N_CORES // 2) * SLOTS   # table half boundary (row 25088)
CAP_HALF = 1280            # pass-2 per-(block, half) edge cap
F1 = 256                   # input features
H1, C1 = 8, 32             # layer-1 heads x channels
R1 = F1 + 2 * H1           # 272: layer-1 row payload  h | s | d
W1ROW = 384                # layer-1 row stride in bf16 elems (768 B)
NCLS = 40
R2 = NCLS + 2              # 42: layer-2 row payload
W2ROW = 128                # layer-2 row stride in bf16 elems (256 B)
NEG_SLOPE = 0.2
TAIL = 999.0               # dst-slot sentinel for pad edge slots

_CACHE: dict = {}


# ============================ host preprocessing ============================

def _greedy_pack(items, weights_list, caps_list, slot_caps):
    """Place items (ordered) into bins; weights_list/caps_list are parallel
    lists of per-item weight arrays and per-bin capacity arrays.  Returns
    bin_of_item.  Greedy: emptiest bin (by total weight) first, skipping
    bins where any cap or the slot cap would overflow."""
    import heapq

    n_bins = len(slot_caps)
    used = [np.zeros(n_bins, dtype=np.int64) for _ in weights_list]
    slots_used = np.zeros(n_bins, dtype=np.int64)
    total = np.zeros(n_bins, dtype=np.int64)
    bin_of = {}
    heap = [(0, b) for b in range(n_bins)]
    heapq.heapify(heap)
    for it in items:
        ws = [w[it] for w in weights_list]
        stash = []
        while True:
            if not heap:
                raise RuntimeError("packing failed; raise CAP_HALF")
            t, b = heapq.heappop(heap)
            if t != total[b]:
                continue  # stale
            if slots_used[b] >= slot_caps[b]:
                continue  # permanently full
            if any(
                used[k][b] + ws[k] > caps_list[k][b] for k in range(len(ws))
            ):
                stash.append((t, b))
                continue
            bin_of[it] = b
            slots_used[b] += 1
            for k in range(len(ws)):
                used[k][b] += ws[k]
            total[b] += sum(ws)
            heapq.heappush(heap, (int(total[b]), b))
            break
        for item in stash:
            heapq.heappush(heap, item)
    return bin_of


def _wrap_idx(lin):
    """Linear index array [n] (n % 16 == 0) -> dma_gather layout
    [128, n // 16] int16 (16-partition wrap, replicated to 128)."""
    n = lin.size
    assert n % 16 == 0
    w = lin.reshape(n // 16, 16).T.astype(np.int16)  # [16, n/16]
    return np.ascontiguousarray(np.tile(w, (8, 1)))  # [128, n/16]


def _ceil16(x):
    return max((int(x) + 15) // 16 * 16, 16)


def _pack_graph(src, dst):
    """Assign nodes to (core, block, slot); route real edges (no self
    loops).  Returns perm_row [N] plus per-core device arrays and the
    baked per-block geometry (max over cores)."""
    deg = np.bincount(dst, minlength=N_NODES)

    # ---- pass 1: nodes -> cores, balancing total in-edges ----
    order = np.argsort(-deg, kind="stable")
    core_of = _greedy_pack(
        order,
        [deg],
        [np.full(N_CORES, 1 << 60, dtype=np.int64)],
        np.full(N_CORES, SLOTS, dtype=np.int64),
    )
    node_core = np.empty(N_NODES, dtype=np.int64)
    for nd, c in core_of.items():
        node_core[nd] = c

    half_b_src = node_core[src] >= (N_CORES // 2)
    degA = np.bincount(dst[~half_b_src], minlength=N_NODES)
    degB = np.bincount(dst[half_b_src], minlength=N_NODES)

    # ---- pass 2: per core, nodes -> blocks with per-half edge caps ----
    node_bin = np.empty(N_NODES, dtype=np.int64)
    node_slot = np.zeros(N_NODES, dtype=np.int64)
    for c in range(N_CORES):
        nodes_c = np.where(node_core == c)[0]
        ordc = nodes_c[np.argsort(-(deg[nodes_c]), kind="stable")]
        bin_of = _greedy_pack(
            ordc,
            [degA, degB],
            [
                np.full(NB, CAP_HALF, dtype=np.int64),
                np.full(NB, CAP_HALF, dtype=np.int64),
            ],
            np.full(NB, PB, dtype=np.int64),
        )
        # relabel bins by descending edge count so block b has similar
        # size on every core (per-block counts are baked as cross-core
        # maxima; aligned quantiles keep the padding small)
        btot = np.zeros(NB, dtype=np.int64)
        for nd in ordc:
            btot[bin_of[nd]] += deg[nd]
        rank = np.empty(NB, dtype=np.int64)
        rank[np.argsort(-btot, kind="stable")] = np.arange(NB)
        slots_used = np.zeros(NB, dtype=np.int64)
        for nd in ordc:
            b = rank[bin_of[nd]]
            node_bin[nd] = c * NB + b
            node_slot[nd] = slots_used[b]
            slots_used[b] += 1

    perm_row = (node_bin * PB + node_slot).astype(np.int64)

    # ---- edge routing: per (bin, half), sorted by src row ----
    n_bins = N_CORES * NB
    ebin = node_bin[dst]
    src_row_e = perm_row[src]
    dst_slot_e = perm_row[dst] % PB
    keyhalf = half_b_src.astype(np.int64)
    sort_idx = np.lexsort((src_row_e, keyhalf, ebin))
    ebin_s = ebin[sort_idx]
    half_s = keyhalf[sort_idx]
    src_s = src_row_e[sort_idx]
    dsl_s = dst_slot_e[sort_idx]

    grp = ebin_s * 2 + half_s
    counts = np.bincount(grp, minlength=n_bins * 2)
    realA = counts[0::2].reshape(N_CORES, NB)
    realB = counts[1::2].reshape(N_CORES, NB)
    assert realA.max() <= CAP_HALF and realB.max() <= CAP_HALF

    # baked per-block geometry: max over cores, ceil 16
    NAb = np.array([_ceil16(realA[:, b].max()) for b in range(NB)])
    NBb = np.array([_ceil16(realB[:, b].max()) for b in range(NB)])
    TbA = (NAb + PB - 1) // PB
    TbB = (NBb + PB - 1) // PB
    Tb = TbA + TbB
    TAmax = int(TbA.max())
    TBmax = int(TbB.max())
    Tmax = int(Tb.max())

    starts = np.zeros(n_bins * 2 + 1, dtype=np.int64)
    np.cumsum(counts, out=starts[1:])
    pos = np.arange(ebin_s.size) - starts[grp]

    b_of_bin = np.arange(n_bins) % NB
    # linear edge slot j within the block: A at [0, NAb), B at TbA*128 +
    j = np.where(half_s == 0, pos, (TbA[b_of_bin] * PB)[ebin_s] + pos)

    linA = np.zeros((n_bins, TAmax * PB), dtype=np.int64)
    linB = np.zeros((n_bins, TBmax * PB), dtype=np.int64)
    dlocF = np.full((n_bins, Tmax * PB), TAIL, dtype=np.float32)

    mA = half_s == 0
    linA[ebin_s[mA], pos[mA]] = src_s[mA]
    linB[ebin_s[~mA], pos[~mA]] = src_s[~mA] - SPLIT
    dlocF[ebin_s, j] = dsl_s

    WA = max(NAb) // 16
    WB = max(NBb) // 16
    idxA = np.zeros((N_CORES, NB, PB, WA), dtype=np.int16)
    idxB = np.zeros((N_CORES, NB, PB, WB), dtype=np.int16)
    for bi in range(n_bins):
        c, b = bi // NB, bi % NB
        wa = _wrap_idx(linA[bi, : NAb[b]])
        idxA[c, b, :, : wa.shape[1]] = wa
        wb = _wrap_idx(linB[bi, : NBb[b]])
        idxB[c, b, :, : wb.shape[1]] = wb

    # dloc [c, b, p, t] = dlocF[c, b, t*128 + p]
    dloc = np.ascontiguousarray(
        dlocF.reshape(N_CORES, NB, Tmax, PB).transpose(0, 1, 3, 2)
    ).astype(BF16)
    dlocF = dlocF.reshape(N_CORES, NB, Tmax * PB).astype(BF16)

    geom = {
        "NAb": tuple(int(x) for x in NAb),
        "NBb": tuple(int(x) for x in NBb),
        "TbA": tuple(int(x) for x in TbA),
        "TbB": tuple(int(x) for x in TbB),
        "Tb": tuple(int(x) for x in Tb),
        "TAmax": TAmax,
        "TBmax": TBmax,
        "Tmax": Tmax,
        "WA": WA,
        "WB": WB,
    }
    return perm_row, idxA, idxB, dloc, dlocF, geom


def _expand_heads(a):
    """[H, C] attention vector -> block-diagonal [H*C, H] matrix."""
    h, c = a.shape
    m = np.zeros((h * c, h), dtype=np.float32)
    for i in range(h):
        m[i * c:(i + 1) * c, i] = a[i]
    return m


# ============================ device program ================================

def _build_program(geom):
    import concourse.bacc as bacc
    import concourse.mybir as mybir
    import concourse.tile as tile

    f32 = mybir.dt.float32
    bf16 = mybir.dt.bfloat16
    i16 = mybir.dt.int16
    Alu = mybir.AluOpType
    Act = mybir.ActivationFunctionType

    NAb, NBb = geom["NAb"], geom["NBb"]
    TbA, TbB, Tb = geom["TbA"], geom["TbB"], geom["Tb"]
    Tmax = geom["Tmax"]
    WA, WB = geom["WA"], geom["WB"]
    single_packet = bool(int(os.environ.get("GAT_SP", "0")))
    MCH = 4                    # BmT build chunk; 4*128 = 512 = max mm free dim
    DEPTH = int(os.environ.get("GAT_DEPTH", "4"))

    nc = bacc.Bacc(
        "TRN2", target_bir_lowering=False, debug=False, num_devices=N_CORES
    )

    # ---- kernel I/O ----
    xT = nc.dram_tensor("xT", [F1, SLOTS], bf16, kind="ExternalInput")
    w1cat = nc.dram_tensor("w1cat", [F1, R1], bf16, kind="ExternalInput")
    w2cat = nc.dram_tensor("w2cat", [F1, R2], bf16, kind="ExternalInput")
    iota_in = nc.dram_tensor("iota_row", [PB, PB], bf16, kind="ExternalInput")
    iotaP_in = nc.dram_tensor("iota_col", [PB, PB], f32, kind="ExternalInput")
    ident_in = nc.dram_tensor("ident", [PB, PB], bf16, kind="ExternalInput")
    idxA_in = nc.dram_tensor("idxA", [NB, PB, WA], i16, kind="ExternalInput")
    idxB_in = nc.dram_tensor("idxB", [NB, PB, WB], i16, kind="ExternalInput")
    dloc_in = nc.dram_tensor("dloc", [NB, PB, Tmax], bf16, kind="ExternalInput")
    dlocF_in = nc.dram_tensor(
        "dlocF", [NB, Tmax * PB], bf16, kind="ExternalInput"
    )
    out_dev = nc.dram_tensor("out_dev", [SLOTS, NCLS], f32, kind="ExternalOutput")

    # ---- internal DRAM ----
    hcat1_own = nc.dram_tensor("hcat1_own", [SLOTS, W1ROW], bf16, kind="Internal")
    hcat1_all = nc.dram_tensor(
        "hcat1_all", [V, W1ROW], bf16, kind="Internal", addr_space="Shared"
    )
    hcat2_own = nc.dram_tensor("hcat2_own", [SLOTS, W2ROW], bf16, kind="Internal")
    hcat2_all = nc.dram_tensor(
        "hcat2_all", [V, W2ROW], bf16, kind="Internal", addr_space="Shared"
    )

    groups = [list(range(N_CORES))]

    with tile.TileContext(nc) as tc:
        with (
            tc.tile_pool(name="persist", bufs=1) as pp,
            tc.tile_pool(name="sb", bufs=2) as sb,
            tc.tile_pool(name="psA", bufs=2, space="PSUM") as psA,
            tc.tile_pool(name="psB", bufs=2, space="PSUM") as psB,
        ):
            # ---------------- persistent tiles ----------------
            iota_sb = pp.tile([PB, PB], bf16, tag="iota")
            nc.sync.dma_start(out=iota_sb[:], in_=iota_in[:, :])
            iotaP_sb = pp.tile([PB, PB], f32, tag="iotaP")
            nc.sync.dma_start(out=iotaP_sb[:], in_=iotaP_in[:, :])
            ident_sb = pp.tile([PB, PB], bf16, tag="ident")
            nc.sync.dma_start(out=ident_sb[:], in_=ident_in[:, :])
            ones_sb = pp.tile([1, PB], bf16, tag="ones")
            nc.vector.memset(ones_sb[:], 1.0)

            w1_sb = [
                pp.tile([PB, R1], bf16, tag=f"w1_{k}", name=f"w1_sb{k}")
                for k in range(2)
            ]
            for k in range(2):
                nc.sync.dma_start(out=w1_sb[k][:], in_=w1cat[k * PB:(k + 1) * PB, :])
            w2_sb = [
                pp.tile([PB, R2], bf16, tag=f"w2_{k}", name=f"w2_sb{k}")
                for k in range(2)
            ]
            for k in range(2):
                nc.sync.dma_start(out=w2_sb[k][:], in_=w2cat[k * PB:(k + 1) * PB, :])

            xT_sb = [
                pp.tile([PB, SLOTS], bf16, tag=f"xT{k}", name=f"xT_sb{k}")
                for k in range(2)
            ]
            for k in range(2):
                nc.sync.dma_start(out=xT_sb[k][:], in_=xT[k * PB:(k + 1) * PB, :])
            h2T_sb = [
                pp.tile([PB, SLOTS], bf16, tag=f"h2T{k}", name=f"h2T_sb{k}")
                for k in range(2)
            ]

            def phase_c_block(nb_i):
                ps = psA.tile([PB, R1], f32, tag="mm")
                for k in range(2):
                    nc.tensor.matmul(
                        out=ps[:][:, 0:R2],
                        lhsT=h2T_sb[k][:][:, nb_i * PB:(nb_i + 1) * PB],
                        rhs=w2_sb[k][:],
                        start=(k == 0),
                        stop=(k == 1),
                    )
                hc2 = sb.tile([PB, R2], bf16, tag="hc2")
                nc.scalar.copy(out=hc2[:], in_=ps[:][:, 0:R2])
                nc.sync.dma_start(
                    out=hcat2_own[nb_i * PB:(nb_i + 1) * PB, 0:R2],
                    in_=hc2[:],
                )

            def edge_layer(layer):
                if layer == 1:
                    table_all, table_own = hcat1_all, hcat1_own
                    WROW, RP, NF, NH = W1ROW, R1, F1, H1
                else:
                    table_all, table_own = hcat2_all, hcat2_own
                    WROW, RP, NF, NH = W2ROW, R2, NCLS, 1
                sfx = f"L{layer}"
                for b in range(NB):
                    tba, tbb, tb = TbA[b], TbB[b], Tb[b]
                    na, nb_ = NAb[b], NBb[b]
                    iA = sb.tile([PB, WA], i16, tag="iA", bufs=DEPTH)
                    nc.sync.dma_start(out=iA[:], in_=idxA_in[b, :, :])
                    iB = sb.tile([PB, WB], i16, tag="iB", bufs=DEPTH)
                    nc.sync.dma_start(out=iB[:], in_=idxB_in[b, :, :])
                    dl = sb.tile([PB, Tmax], bf16, tag="dl", bufs=DEPTH)
                    nc.sync.dma_start(out=dl[:], in_=dloc_in[b, :, :])
                    dlF = sb.tile([1, Tmax * PB], bf16, tag="dlF", bufs=DEPTH)
                    nc.sync.dma_start(out=dlF[:], in_=dlocF_in[b:b + 1, :])
                    own = sb.tile([PB, RP], bf16, tag="own" + sfx, bufs=DEPTH)
                    nc.sync.dma_start(
                        out=own[:], in_=table_own[b * PB:(b + 1) * PB, 0:RP]
                    )

                    G = sb.tile(
                        [PB, Tmax * WROW], bf16, tag="G" + sfx, bufs=DEPTH
                    )
                    G3 = G[:].rearrange("p (t f) -> p t f", t=Tmax)
                    # zero the partial tail tiles (junk killed by zero
                    # one-hot columns, but must stay finite)
                    nc.vector.memset(G3[:, tba - 1, :], 0.0)
                    nc.vector.memset(G3[:, tb - 1, :], 0.0)
                    nc.gpsimd.dma_gather(
                        out_ap=G3[:, 0:tba, :],
                        in_ap=table_all[0:SPLIT, :],
                        idxs_ap=iA[:][:, 0:na // 16],
                        num_idxs=na,
                        num_idxs_reg=na,
                        elem_size=WROW,
                        single_packet=single_packet,
                    )
                    nc.gpsimd.dma_gather(
                        out_ap=G3[:, tba:tb, :],
                        in_ap=table_all[SPLIT:V, :],
                        idxs_ap=iB[:][:, 0:nb_ // 16],
                        num_idxs=nb_,
                        num_idxs_reg=nb_,
                        elem_size=WROW,
                        single_packet=single_packet,
                    )

                    # one-hot Bm[e, (t, d)] for aggregation
                    Bm = sb.tile([PB, Tmax * PB], bf16, tag="Bm", bufs=DEPTH)
                    Bm3 = Bm[:].rearrange("p (t d) -> p t d", t=Tmax)
                    nc.vector.tensor_tensor(
                        out=Bm3[:, 0:tb, :],
                        in0=dl[:][:, 0:tb, None].broadcast_to([PB, tb, PB]),
                        in1=iota_sb[:][:, None, :].broadcast_to([PB, tb, PB]),
                        op=Alu.is_equal,
                    )

                    # transposed one-hot BmT[d, (t, e)]
                    BmT = sb.tile([PB, Tmax * PB], bf16, tag="BmT", bufs=DEPTH)
                    BmT3 = BmT[:].rearrange("p (t d) -> p t d", t=Tmax)
                    for c0 in range(0, tb, MCH):
                        ch = min(MCH, tb - c0)
                        Mc = psB.tile([PB, MCH * PB], f32, tag="Mc")
                        nc.tensor.matmul(
                            out=Mc[:][:, 0:ch * PB],
                            lhsT=ones_sb[:],
                            rhs=dlF[:][:, c0 * PB:(c0 + ch) * PB],
                            start=True,
                            stop=True,
                        )
                        Mc3 = Mc[:].rearrange("p (t d) -> p t d", t=MCH)
                        nc.vector.tensor_tensor(
                            out=BmT3[:, c0:c0 + ch, :],
                            in0=Mc3[:, 0:ch, :],
                            in1=iotaP_sb[:][:, None, :].broadcast_to(
                                [PB, ch, PB]
                            ),
                            op=Alu.is_equal,
                        )

                    # alpha_dst per edge slot: D2[e, (t, h)] = BmT_t.T @ dblk
                    D2 = psB.tile([PB, (Tmax + 1) * NH], f32, tag="D2")
                    for t in range(tb):
                        nc.tensor.matmul(
                            out=D2[:][:, t * NH:(t + 1) * NH],
                            lhsT=BmT[:][:, t * PB:(t + 1) * PB],
                            rhs=own[:][:, NF + NH:NF + 2 * NH],
                            start=True,
                            stop=True,
                        )

                    # logits -> p = exp(leaky_relu(s_src + d_dst))
                    sf = sb.tile([PB, Tmax * NH], f32, tag="sf", bufs=DEPTH)
                    sf3 = sf[:].rearrange("p (t h) -> p t h", t=Tmax)
                    nc.scalar.copy(
                        out=sf3[:, 0:tb, :], in_=G3[:, 0:tb, NF:NF + NH]
                    )
                    lg = sb.tile([PB, Tmax * NH], f32, tag="lg", bufs=DEPTH)
                    nc.vector.tensor_tensor(
                        out=lg[:][:, 0:tb * NH],
                        in0=sf[:][:, 0:tb * NH],
                        in1=D2[:][:, 0:tb * NH],
                        op=Alu.add,
                    )
                    nc.scalar.activation(
                        out=lg[:][:, 0:tb * NH], in_=lg[:][:, 0:tb * NH],
                        func=Act.Prelu, alpha=NEG_SLOPE,
                    )
                    p = sb.tile([PB, Tmax * NH], bf16, tag="p", bufs=DEPTH)
                    nc.scalar.activation(
                        out=p[:][:, 0:tb * NH], in_=lg[:][:, 0:tb * NH],
                        func=Act.Exp,
                    )
                    p3 = p[:].rearrange("p (t h) -> p t h", t=Tmax)

                    # in-place: G[:, :, 0:NF] *= p
                    out4 = G3[:, 0:tb, 0:NF].rearrange(
                        "p t (h c) -> p t h c", h=NH
                    )
                    nc.vector.tensor_tensor(
                        out=out4,
                        in0=out4,
                        in1=p3[:, 0:tb, :, None].broadcast_to(
                            [PB, tb, NH, NF // NH]
                        ),
                        op=Alu.mult,
                    )

                    # accumulate out[d] = B.T @ (p*h) and den[d] = B.T @ p
                    po = psA.tile([PB, R1], f32, tag="mm")
                    for t in range(tb):
                        nc.tensor.matmul(
                            out=po[:][:, 0:NF],
                            lhsT=Bm[:][:, t * PB:(t + 1) * PB],
                            rhs=G3[:, t, 0:NF],
                            start=(t == 0),
                            stop=(t == tb - 1),
                        )
                    for t in range(tb):
                        nc.tensor.matmul(
                            out=D2[:][:, Tmax * NH:(Tmax + 1) * NH],
                            lhsT=Bm[:][:, t * PB:(t + 1) * PB],
                            rhs=p[:][:, t * NH:(t + 1) * NH],
                            start=(t == 0),
                            stop=(t == tb - 1),
                        )

                    # self loop: p_self = exp(leaky_relu(s_own + d_own))
                    sd = sb.tile([PB, NH], f32, tag="sd")
                    nc.vector.tensor_tensor(
                        out=sd[:], in0=own[:][:, NF:NF + NH],
                        in1=own[:][:, NF + NH:NF + 2 * NH], op=Alu.add,
                    )
                    nc.scalar.activation(
                        out=sd[:], in_=sd[:], func=Act.Prelu, alpha=NEG_SLOPE
                    )
                    pself = sb.tile([PB, NH], f32, tag="pself")
                    nc.scalar.activation(out=pself[:], in_=sd[:], func=Act.Exp)
                    pselfb = sb.tile([PB, NH], bf16, tag="pselfb")
                    nc.scalar.copy(out=pselfb[:], in_=pself[:])

                    of = sb.tile([PB, NF], f32, tag="of" + sfx)
                    nc.scalar.copy(out=of[:], in_=po[:][:, 0:NF])
                    slh = sb.tile([PB, NF], f32, tag="slh" + sfx)
                    slh3 = slh[:].rearrange("p (h c) -> p h c", h=NH)
                    nc.vector.tensor_tensor(
                        out=slh3,
                        in0=own[:][:, 0:NF].rearrange("p (h c) -> p h c", h=NH),
                        in1=pselfb[:][:, :, None].broadcast_to(
                            [PB, NH, NF // NH]
                        ),
                        op=Alu.mult,
                    )
                    nc.vector.tensor_tensor(
                        out=of[:], in0=of[:], in1=slh[:], op=Alu.add,
                    )
                    denf = sb.tile([PB, NH], f32, tag="denf")
                    nc.vector.tensor_tensor(
                        out=denf[:],
                        in0=D2[:][:, Tmax * NH:(Tmax + 1) * NH],
                        in1=pself[:], op=Alu.add,
                    )
                    rden = sb.tile([PB, NH], f32, tag="rden")
                    nc.vector.reciprocal(out=rden[:], in_=denf[:])

                    o1 = sb.tile([PB, NF], f32, tag="o1" + sfx)
                    o13 = o1[:].rearrange("p (h c) -> p h c", h=NH)
                    nc.vector.tensor_tensor(
                        out=o13,
                        in0=of[:].rearrange("p (h c) -> p h c", h=NH),
                        in1=rden[:][:, :, None].broadcast_to(
                            [PB, NH, NF // NH]
                        ),
                        op=Alu.mult,
                    )

                    if layer == 1:
                        # elu(x) = max(x,0) - 1 + exp(min(x,0)) -> h2 bf16
                        mneg = sb.tile([PB, NF], f32, tag="mneg")
                        nc.vector.tensor_scalar_min(
                            out=mneg[:], in0=o1[:], scalar1=0.0
                        )
                        eneg = sb.tile([PB, NF], f32, tag="eneg")
                        nc.scalar.activation(
                            out=eneg[:], in_=mneg[:], func=Act.Exp
                        )
                        h2a = sb.tile([PB, NF], f32, tag="h2a")
                        nc.vector.tensor_scalar(
                            out=h2a[:], in0=o1[:], scalar1=0.0, scalar2=-1.0,
                            op0=Alu.max, op1=Alu.add,
                        )
                        h2 = sb.tile([PB, NF], bf16, tag="h2")
                        nc.vector.tensor_tensor(
                            out=h2[:], in0=h2a[:], in1=eneg[:], op=Alu.add
                        )
                        for k in range(2):
                            pt = psB.tile([PB, PB], bf16, tag="tr")
                            nc.tensor.transpose(
                                out=pt[:],
                                in_=h2[:][:, k * PB:(k + 1) * PB],
                                identity=ident_sb[:],
                            )
                            nc.scalar.copy(
                                out=h2T_sb[k][:][:, b * PB:(b + 1) * PB],
                                in_=pt[:],
                            )
                        phase_c_block(b)
                    else:
                        nc.sync.dma_start(
                            out=out_dev[b * PB:(b + 1) * PB, :],
                            in_=o1[:][:, 0:NCLS],
                        )

            with nc.named_scope("gat"):
                # ---------------- phase A: hcat1 = x @ W1cat ----------------
                for nb_i in range(NB):
                    ps = psA.tile([PB, R1], f32, tag="mm")
                    for k in range(2):
                        nc.tensor.matmul(
                            out=ps[:],
                            lhsT=xT_sb[k][:][:, nb_i * PB:(nb_i + 1) * PB],
                            rhs=w1_sb[k][:],
                            start=(k == 0),
                            stop=(k == 1),
                        )
                    hc = sb.tile([PB, R1], bf16, tag="hc1")
                    nc.scalar.copy(out=hc[:], in_=ps[:])
                    nc.sync.dma_start(
                        out=hcat1_own[nb_i * PB:(nb_i + 1) * PB, 0:R1],
                        in_=hc[:],
                    )

                nc.gpsimd.collective_compute(
                    "AllGather",
                    mybir.AluOpType.bypass,
                    replica_groups=groups,
                    ins=[hcat1_own[:, :].opt()],
                    outs=[hcat1_all[:, :].opt()],
                )

                edge_layer(1)   # phase C interleaved per block

                nc.gpsimd.collective_compute(
                    "AllGather",
                    mybir.AluOpType.bypass,
                    replica_groups=groups,
                    ins=[hcat2_own[:, :].opt()],
                    outs=[hcat2_all[:, :].opt()],
                )

                edge_layer(2)

    nc.compile()
    return nc


# ============================ top-level entry ===============================

def _prepare(inputs):
    x = np.ascontiguousarray(np.asarray(inputs["x"], dtype=np.float32))
    edge_index = np.asarray(inputs["edge_index"], dtype=np.int64)
    w1 = np.asarray(inputs["w1"], dtype=np.float32)
    a_src1 = np.asarray(inputs["a_src1"], dtype=np.float32)
    a_dst1 = np.asarray(inputs["a_dst1"], dtype=np.float32)
    b1 = np.asarray(inputs["b1"], dtype=np.float32)
    w2 = np.asarray(inputs["w2"], dtype=np.float32)
    a_src2 = np.asarray(inputs["a_src2"], dtype=np.float32)
    a_dst2 = np.asarray(inputs["a_dst2"], dtype=np.float32)
    b2 = np.asarray(inputs["b2"], dtype=np.float32)

    assert x.shape == (N_NODES, F1) and edge_index.shape == (2, N_EDGES)
    assert np.all(np.abs(b1) == 0.0), "kernel hardcodes b1 == 0"

    src = edge_index[0]
    dst = edge_index[1]
    perm_row, idxA, idxB, dloc, dlocF, geom = _pack_graph(src, dst)

    w1cat = np.concatenate(
        [w1, w1 @ _expand_heads(a_src1), w1 @ _expand_heads(a_dst1)], axis=1
    ).astype(BF16)
    w2cat = np.concatenate(
        [w2, w2 @ _expand_heads(a_src2), w2 @ _expand_heads(a_dst2)], axis=1
    ).astype(BF16)

    xp = np.zeros((V, F1), dtype=np.float32)
    xp[perm_row] = x
    iota_f = np.broadcast_to(np.arange(PB, dtype=np.float32), (PB, PB))
    iota_row = np.ascontiguousarray(iota_f).astype(BF16)
    iota_col = np.ascontiguousarray(iota_f.T)
    ident = np.eye(PB, dtype=np.float32).astype(BF16)

    xpb = xp.astype(BF16)
    in_maps = []
    for c in range(N_CORES):
        xT_c = np.ascontiguousarray(xpb[c * SLOTS:(c + 1) * SLOTS].T)
        in_maps.append(
            {
                "xT": xT_c,
                "w1cat": w1cat,
                "w2cat": w2cat,
                "iota_row": iota_row,
                "iota_col": iota_col,
                "ident": ident,
                "idxA": idxA[c],
                "idxB": idxB[c],
                "dloc": dloc[c],
                "dlocF": dlocF[c],
            }
        )
    return in_maps, perm_row, b2, geom


def _assemble(core_outs, perm_row, b2):
    out_all = np.concatenate(core_outs, axis=0)
    out = out_all[perm_row] + b2[None, :]
    return out.astype(np.float32)


def kernel(**inputs) -> np.ndarray:
    in_maps, perm_row, b2, geom = _prepare(inputs)

    import concourse.bass_utils as bass_utils

    key = ("nc", tuple(sorted(geom.items())))
    if key not in _CACHE:
        _CACHE.clear()
        _CACHE[key] = _build_program(geom)
    nc = _CACHE[key]

    trace = bool(int(os.environ.get("GAT_TRACE", "0")))
    res = bass_utils.run_bass_kernel_spmd(
        nc,
        in_maps,
        core_ids=list(range(N_CORES)),
        trace=trace,
        trace_cores=list(range(N_CORES)) if trace else None,
        stitch_traces=trace,
    )
    _CACHE["last_results"] = res

    return _assemble([r["out_dev"] for r in res.results], perm_row, b2)
